# revision 1
# baseline (speedup 1.0000x reference)
"""Multi-head causal attention (B=4,T=2048,C=1024,H=16,D=64) on 8 TRN2 NeuronCores.

Sharding: no collectives. Core c handles batch b=c//2 and a causally-balanced
set of four 256-query chunks (half=c%2): half0 -> chunks [7,5,2,0], half1 ->
[6,4,3,1]. Every core runs the same program with padded per-slot key-tile
counts [16,12,8,4]; per-core differences (real counts / causal diagonals) are
expressed purely through per-core input data (mask tiles). K/V projections are
computed for the full sequence on both cores of a batch (duplication instead
of cross-core communication, which is far slower on this part).

Pipeline per core (one Bass/Tile program):
  B-stage: qT = (x @ Wq)^T for the core's 1024 query columns -> SBUF resident.
  A-stage: kT = (x @ Wk)^T -> DRAM scratch; v = x @ Wv (+ones col) -> DRAM.
  C-stage: per head-pair, per key tile j: scores = kT_j^T @ qT (fp32r,
           2 heads packed via PE row groups, separate PSUM banks), exp on ACT
           (scale folded in), causal/pad masks multiplied on the one slot that
           needs them, wei @ [v|1] accumulated in PSUM ([65,256] per head:
           row 64 = sumexp). Normalize with DVE fast reciprocal + GPSIMD
           partition broadcast.
  proj:    out = attn^T-layout tiles @ Wp (+bias), written per chunk.
All matmuls run as float32r with moving free dim >= 256 (full PE rate).
"""

import numpy as np

import concourse.bass as bass
import concourse.tile as tile
from concourse import bacc, library_config, mybir
from concourse.bass_utils import run_bass_kernel_spmd

B, T, C = 4, 2048, 1024
H, D = 16, 64
P = 128            # key tile size
QC = 256           # query chunk size
NP = 8             # head pairs
PN = [16, 12, 8, 4]                    # padded per-slot key-tile counts
CHUNKS = [[7, 5, 2, 0], [6, 4, 3, 1]]  # chunk ids per half, slot order
F32 = mybir.dt.float32
F32R = mybir.dt.float32r
EXP = mybir.ActivationFunctionType.Exp
SCALE = float(C) ** -0.5


def r(ap):
    """fp32 AP -> fp32r view for full-rate PE matmuls."""
    return ap.bitcast(F32R)


def build_kernel(nc: bass.Bass):
    xT = nc.dram_tensor("xT", [C, T], F32, kind="ExternalInput").ap()
    xq = nc.dram_tensor("xq", [C, 4 * QC], F32, kind="ExternalInput").ap()
    wq2 = nc.dram_tensor("wq2", [C, C], F32, kind="ExternalInput").ap()
    wk2 = nc.dram_tensor("wk2", [C, C], F32, kind="ExternalInput").ap()
    wv2 = nc.dram_tensor("wv2", [C, C], F32, kind="ExternalInput").ap()
    wp = nc.dram_tensor("wp", [C, C], F32, kind="ExternalInput").ap()
    bias = nc.dram_tensor("bias", [1, C], F32, kind="ExternalInput").ap()
    masks = nc.dram_tensor("masks", [16, P, QC], F32, kind="ExternalInput").ap()
    eye = nc.dram_tensor("eye", [P, P], F32, kind="ExternalInput").ap()
    out = nc.dram_tensor("out", [4, QC, C], F32, kind="ExternalOutput").ap()

    kT_d = nc.dram_tensor("kT_scratch", [C, T], F32).ap()
    v_d = nc.dram_tensor("v_scratch", [16, P, 16 * 65], F32).ap()

    with tile.TileContext(nc) as tc:
        nc.gpsimd.load_library(library_config.attn)
        with tc.tile_pool(name="const", bufs=1) as cpool:
            qT_sb = cpool.tile([P, NP * 1024], F32R)
            attn_sb = cpool.tile([P, NP * 1024], F32R)
            masks_sb = cpool.tile([P, 16 * QC], F32R)
            eye_sb = cpool.tile([P, P], F32R)

            # ---------------- B then A projections ----------------
            with (
                tc.tile_pool(name="ps_mm", bufs=2, space="PSUM") as ps_mm,
                tc.tile_pool(name="xt", bufs=8) as xtpool,
                tc.tile_pool(name="stage", bufs=2) as stpool,
                tc.tile_pool(name="vstage", bufs=1) as vstpool,
            ):
                with tc.tile_pool(name="wqp", bufs=1) as wqpool:
                    wq_sb = wqpool.tile([P, 8 * 1024], F32R, tag="wq")
                    for g in range(8):
                        nc.sync.dma_start(
                            wq_sb[:, g * 1024:(g + 1) * 1024],
                            wq2[g * P:(g + 1) * P, :].bitcast(F32R),
                        )
                    for k in range(4):
                        xqg = [
                            xtpool.tile([P, QC], F32R, tag="xt0", name=f"xq{g}")
                            for g in range(8)
                        ]
                        for g in range(8):
                            nc.sync.dma_start(
                                xqg[g][:],
                                xq[g * P:(g + 1) * P,
                                   k * QC:(k + 1) * QC].bitcast(F32R),
                            )
                        for p in range(NP):
                            qps = ps_mm.tile([P, QC], F32, tag="mm")
                            for g in range(8):
                                nc.tensor.matmul(
                                    qps[:],
                                    r(wq_sb[:, g * 1024 + (2 * p) * 64:][:, :128]),
                                    xqg[g][:],
                                    start=(g == 0), stop=(g == 7),
                                )
                            nc.scalar.copy(
                                qT_sb[:, p * 1024 + k * QC:][:, :QC], qps[:]
                            )

                with tc.tile_pool(name="wkvp", bufs=1) as wkvpool:
                    wk_sb = wkvpool.tile([P, 8 * 1024], F32R, tag="wk")
                    wv_sb = wkvpool.tile([P, 8 * 1024], F32R, tag="wv")
                    for g in range(8):
                        gs = slice(g * 1024, (g + 1) * 1024)
                        nc.sync.dma_start(
                            wk_sb[:, gs], wk2[g * P:(g + 1) * P, :].bitcast(F32R)
                        )
                        nc.sync.dma_start(
                            wv_sb[:, gs], wv2[g * P:(g + 1) * P, :].bitcast(F32R)
                        )
                    for tb in range(4):
                        ts_ = slice(tb * 512, (tb + 1) * 512)
                        xtg = [
                            xtpool.tile([P, 512], F32R, tag=f"xt{tb % 2}",
                                        name=f"xt{g}")
                            for g in range(8)
                        ]
                        for g in range(8):
                            nc.sync.dma_start(
                                xtg[g][:], xT[g * P:(g + 1) * P, ts_].bitcast(F32R)
                            )
                        for p in range(NP):
                            kps = ps_mm.tile([P, 512], F32, tag="mm")
                            for g in range(8):
                                nc.tensor.matmul(
                                    kps[:],
                                    r(wk_sb[:, g * 1024 + (2 * p) * 64:][:, :128]),
                                    xtg[g][:],
                                    start=(g == 0), stop=(g == 7),
                                )
                            kst = stpool.tile([P, 512], F32, tag="kst")
                            nc.vector.tensor_copy(kst[:], kps[:])
                            nc.sync.dma_start(kT_d[p * P:(p + 1) * P, ts_], kst[:])
                        for sti in range(4):
                            j = tb * 4 + sti
                            vst = vstpool.tile([P, 16 * 65], F32, tag="vst")
                            vv = vst[:].rearrange("p (h e) -> p h e", e=65)
                            nc.vector.memset(vv[:, :, 64:65], 1.0)
                            for hc in range(2):
                                vps = ps_mm.tile([P, 512], F32, tag="mm")
                                for g in range(8):
                                    nc.tensor.matmul(
                                        vps[:],
                                        r(xtg[g][:, sti * P:(sti + 1) * P]),
                                        wv_sb[:, g * 1024 + hc * 512:][:, :512],
                                        start=(g == 0), stop=(g == 7),
                                    )
                                nc.scalar.copy(
                                    vv[:, hc * 8:(hc + 1) * 8, 0:64],
                                    vps[:].rearrange("p (h d) -> p h d", d=64),
                                )
                            nc.sync.dma_start(v_d[j], vst[:])

            # ---------------- C: attention + proj ----------------
            for i in range(16):
                nc.sync.dma_start(
                    masks_sb[:, i * QC:(i + 1) * QC], masks[i].bitcast(F32R)
                )
            nc.sync.dma_start(eye_sb[:], eye[:].bitcast(F32R))
            with (
                tc.tile_pool(name="kv", bufs=2) as kvpool,
                tc.tile_pool(name="exp", bufs=3) as epool,
                tc.tile_pool(name="norm", bufs=1) as npool,
                tc.tile_pool(name="wpp", bufs=1) as wppool,
                tc.tile_pool(name="outp", bufs=3) as outpool,
                tc.tile_pool(name="ps_sc", bufs=2, space="PSUM") as ps_sc,
                tc.tile_pool(name="ps_av", bufs=2, space="PSUM") as ps_av,
                tc.tile_pool(name="ps_pj", bufs=2, space="PSUM") as ps_pj,
            ):
                wp_sb = wppool.tile([P, 8 * 1024], F32R, tag="wp")
                for g in range(8):
                    nc.sync.dma_start(
                        wp_sb[:, g * 1024:(g + 1) * 1024],
                        wp[g * P:(g + 1) * P, :].bitcast(F32R),
                    )
                bias_s = wppool.tile([1, C], F32, tag="bias1")
                nc.sync.dma_start(bias_s[:], bias[:])
                bias_bc = wppool.tile([P, C], F32, tag="biasbc")
                nc.gpsimd.partition_broadcast(bias_bc[:], bias_s[:])

                def c_run(k, p):
                    avp = ps_av.tile([65, 2 * QC], F32, tag="av",
                                     name=f"av{k}_{p}")
                    qA = qT_sb[0:64, p * 1024 + k * QC:][:, :QC]
                    qB = qT_sb[64:128, p * 1024 + k * QC:][:, :QC]
                    njc = PN[k] // 4
                    pend = None  # (e_t, v0, v1, j0) awaiting AV emission

                    def emit_av(pv):
                        e_t, v0, v1, j0 = pv
                        nc.tensor.matmul(avp[:, 0:QC], v0[:, 0:65],
                                         e_t[:, 0:QC],
                                         start=(j0 == 0), stop=False)
                        nc.tensor.matmul(avp[:, 0:QC], v1[:, 0:65],
                                         e_t[:, QC:2 * QC],
                                         start=False, stop=False)
                        nc.tensor.matmul(avp[:, QC:2 * QC], v0[:, 65:130],
                                         e_t[:, 2 * QC:3 * QC],
                                         start=False, stop=False)
                        nc.tensor.matmul(avp[:, QC:2 * QC], v1[:, 65:130],
                                         e_t[:, 3 * QC:4 * QC],
                                         start=False, stop=(j0 + 1 == PN[k] - 1))

                    for jc in range(njc):
                        ktc = kvpool.tile([P, 4 * P], F32R, tag="kt")
                        nc.sync.dma_start(
                            ktc[:],
                            kT_d[p * P:(p + 1) * P,
                                 jc * 4 * P:(jc + 1) * 4 * P].bitcast(F32R),
                        )
                        vc = kvpool.tile([P, 4 * 130], F32R, tag="vt")
                        nc.sync.dma_start(
                            vc[:].rearrange("s (j c) -> s j c", c=130),
                            v_d[4 * jc:4 * jc + 4, :,
                                2 * p * 65:(2 * p + 2) * 65]
                            .rearrange("j s c -> s j c").bitcast(F32R),
                        )
                        for u in range(2):
                            j0 = 4 * jc + 2 * u
                            masked = j0 >= PN[k] - 4
                            kt0 = ktc[:, (2 * u) * P:(2 * u + 1) * P]
                            kt1 = ktc[:, (2 * u + 1) * P:(2 * u + 2) * P]
                            v0 = vc[:, (2 * u) * 130:(2 * u + 1) * 130]
                            v1 = vc[:, (2 * u + 1) * 130:(2 * u + 2) * 130]
                            sc = ps_sc.tile([P, 4 * QC], F32, tag="sc")
                            nc.tensor.matmul(sc[:, 0:QC], r(kt0[0:64, :]), qA,
                                             start=True, stop=False,
                                             tile_position=(0, 0))
                            nc.tensor.matmul(sc[:, 2 * QC:3 * QC],
                                             r(kt0[64:128, :]), qB,
                                             start=True, stop=False,
                                             tile_position=(64, 0))
                            nc.tensor.matmul(sc[:, QC:2 * QC], r(kt1[0:64, :]),
                                             qA, start=False, stop=not masked,
                                             tile_position=(0, 0))
                            nc.tensor.matmul(sc[:, 3 * QC:4 * QC],
                                             r(kt1[64:128, :]), qB,
                                             start=False, stop=not masked,
                                             tile_position=(64, 0))
                            if masked:
                                li = (k * 4 + (j0 - (PN[k] - 4))) * QC
                                mb = masks_sb[:, li:li + 2 * QC]
                                nc.tensor.matmul(sc[:, 0:2 * QC], eye_sb[:], mb,
                                                 start=False, stop=True)
                                nc.tensor.matmul(sc[:, 2 * QC:4 * QC], eye_sb[:],
                                                 mb, start=False, stop=True)
                            e_t = epool.tile([P, 4 * QC], F32R, tag="exp")
                            nc.scalar.activation(e_t[:], sc[:], EXP, scale=SCALE)
                            if pend is not None:
                                emit_av(pend)
                            pend = (e_t, v0, v1, j0)
                    emit_av(pend)
                    rs = npool.tile([1, 2 * QC], F32, tag="rs", bufs=2)
                    nc.vector.tensor_copy(rs[:], avp[64:65, :])
                    avc = npool.tile([64, 2 * QC], F32, tag="avc", bufs=2)
                    nc.vector.tensor_copy(avc[:], avp[0:64, :])
                    rc = npool.tile([1, 2 * QC], F32, tag="rc", bufs=2)
                    nc.vector.reciprocal_approx_fast(rc[:], rs[:])
                    rb = npool.tile([64, 2 * QC], F32, tag="rb", bufs=2)
                    nc.gpsimd.partition_broadcast(rb[:], rc[:])
                    col = p * 1024 + k * QC
                    nc.vector.tensor_mul(attn_sb[0:64, col:col + QC],
                                         avc[:, 0:QC], rb[:, 0:QC])
                    nc.vector.tensor_mul(attn_sb[64:128, col:col + QC],
                                         avc[:, QC:2 * QC], rb[:, QC:2 * QC])

                def proj_unit(k, tt, oc):
                    pp = ps_pj.tile([P, 512], F32, tag="pj")
                    for g in range(NP):
                        nc.tensor.matmul(
                            pp[:],
                            r(attn_sb[:, g * 1024 + k * QC + tt * P:][:, :P]),
                            wp_sb[:, g * 1024 + oc * 512:][:, :512],
                            start=(g == 0), stop=(g == 7),
                        )
                    ot = outpool.tile([P, 512], F32, tag="ot")
                    nc.vector.tensor_add(
                        ot[:], pp[:], bias_bc[:, oc * 512:(oc + 1) * 512]
                    )
                    nc.sync.dma_start(
                        out[k, tt * P:(tt + 1) * P, oc * 512:(oc + 1) * 512],
                        ot[:],
                    )

                for k in (3, 2, 1, 0):
                    for p in range(NP):
                        c_run(k, p)
                    for tt in range(2):
                        for oc in range(2):
                            proj_unit(k, tt, oc)
    return nc


def _make_masks(half):
    chunks = CHUNKS[half]
    m = np.zeros((16, P, QC), np.float32)
    s = np.arange(P)[:, None]
    t = np.arange(QC)[None, :]
    for k in range(4):
        q = chunks[k]
        n = 2 * (q + 1)
        for l in range(4):
            j = PN[k] - 4 + l
            if j >= n:
                pat = np.full((P, QC), -1e6, np.float32)
            elif j == n - 2:
                pat = np.where(s <= t, 0.0, -1e6).astype(np.float32)
            elif j == n - 1:
                pat = np.where(s <= t - 128, 0.0, -1e6).astype(np.float32)
            else:
                pat = np.zeros((P, QC), np.float32)
            m[k * 4 + l] = pat
    return m


_CACHE = {}


def _get_nc():
    if "nc" not in _CACHE:
        nc = bacc.Bacc("TRN2", target_bir_lowering=False, debug=False)
        build_kernel(nc)
        nc.compile()
        _CACHE["nc"] = nc
    return _CACHE["nc"]


def make_in_maps(x, wq, wk, wv, w_proj, b_proj):
    x = np.ascontiguousarray(np.asarray(x, np.float32))
    wq2 = np.ascontiguousarray(np.transpose(np.asarray(wq), (1, 0, 2)).reshape(C, C))
    wk2 = np.ascontiguousarray(np.transpose(np.asarray(wk), (1, 0, 2)).reshape(C, C))
    wv2 = np.ascontiguousarray(np.transpose(np.asarray(wv), (1, 0, 2)).reshape(C, C))
    wpm = np.ascontiguousarray(np.asarray(w_proj, np.float32))
    bias = np.asarray(b_proj, np.float32).reshape(1, C)
    masks_h = [_make_masks(0), _make_masks(1)]

    in_maps = []
    for core in range(8):
        b, half = core // 2, core % 2
        xTb = np.ascontiguousarray(x[b].T)
        xqb = np.ascontiguousarray(
            np.concatenate(
                [xTb[:, q * QC:(q + 1) * QC] for q in CHUNKS[half]], axis=1
            )
        )
        in_maps.append({
            "xT": xTb, "xq": xqb,
            "wq2": wq2, "wk2": wk2, "wv2": wv2,
            "wp": wpm, "bias": bias, "masks": masks_h[half],
            "eye": np.eye(P, dtype=np.float32),
        })
    return in_maps


def assemble(results):
    full = np.zeros((B, T, C), np.float32)
    for core in range(8):
        b, half = core // 2, core % 2
        o = results[core]["out"]
        for k, q in enumerate(CHUNKS[half]):
            full[b, q * QC:(q + 1) * QC] = o[k]
    return full


def kernel(x, wq, wk, wv, w_proj, b_proj, _trace=False, _tmpdir=None):
    in_maps = make_in_maps(x, wq, wk, wv, w_proj, b_proj)
    nc = _get_nc()
    res = run_bass_kernel_spmd(
        nc, in_maps, core_ids=list(range(8)), trace=_trace, tmpdir=_tmpdir
    )
    if _trace:
        _CACHE["last_result"] = res
    return assemble(res.results)



# revision 5
# speedup vs baseline: 1.5598x; 1.5598x over previous
"""Multi-head causal attention (B=4,T=2048,C=1024,H=16,D=64) on 8 TRN2 NeuronCores.

Sharding: batch x head-half tensor parallel. Core c handles batch b=c//2 and
heads [8*(c%2), 8*(c%2)+8) over ALL 2048 queries. Each core computes its own
Q/K/V projections (no duplicated work, no cross-core traffic), causal
attention for its 8 heads, and a partial output projection
attn_half @ w_proj[half_rows]. The host unshards by summing the two partials
per batch and adding the bias (the all-reduce of the TP sharding, done at
gather time). Causal load is uniform per core by construction.

Per-core pipeline (one Bass/Tile program, all engines overlapped, bf16):
  Stage 1: per 512-query block, qT/kT ([hc,t], SBUF-resident) and v
           ([s, hc]+ones cols) projections, contraction over 8 c-groups.
  Stage 2: per (head-pair, 256-query chunk): scores = kT_j^T @ qT for the
           causal prefix of 128-key tiles (2 heads packed via PE row groups),
           exp on ACT (scale folded), 0/1 causal mask applied post-exp on DVE
           for the diagonal unit only, wei @ [v|1] accumulated in PSUM
           ([65,256]/head, row 64 = sumexp). AV emission is delayed one unit
           (global software pipeline) so the PE never waits on ACT.
           Normalize: DVE fast reciprocal + GPSIMD partition broadcast.
  Proj:    partial out = attn^T tiles @ wp, deferred one query-chunk so the
           PE keeps streaming while DVE normalization drains.
All matmuls bf16 (fp32 PSUM accumulate): full PE rate with moving dim >= 256
and 2x faster weight loads (FWL) for the LDW-heavy score/AV stages.
"""

import numpy as np
import ml_dtypes

import concourse.bass as bass
import concourse.tile as tile
from concourse import bacc, library_config, mybir
from concourse.bass_utils import run_bass_kernel_spmd

B, T, C = 4, 2048, 1024
H, D = 16, 64
P = 128           # key tile size
QC = 256          # query chunk size
HC = 512          # head channels per core (8 heads x 64)
NP = 4            # head pairs per core
BF = mybir.dt.bfloat16
F32 = mybir.dt.float32
BF16NP = ml_dtypes.bfloat16
EXP = mybir.ActivationFunctionType.Exp
SCALE = float(C) ** -0.5
VS = 2 * NP * 65  # 520: per key-tile v row: 4 pairs x (2 heads x 65)


def build_kernel(nc: bass.Bass):
    xT = nc.dram_tensor("xT", [C, T], BF, kind="ExternalInput").ap()
    wq = nc.dram_tensor("wq", [C, HC], BF, kind="ExternalInput").ap()
    wk = nc.dram_tensor("wk", [C, HC], BF, kind="ExternalInput").ap()
    wv = nc.dram_tensor("wv", [C, HC], BF, kind="ExternalInput").ap()
    wp = nc.dram_tensor("wp", [HC, C], BF, kind="ExternalInput").ap()
    maskd = nc.dram_tensor("maskd", [P, 4 * QC], BF, kind="ExternalInput").ap()
    outd = nc.dram_tensor("out", [T, C], BF, kind="ExternalOutput").ap()

    with tile.TileContext(nc) as tc:
        nc.gpsimd.load_library(library_config.attn)
        with tc.tile_pool(name="res", bufs=1) as rpool:
            qT_sb = rpool.tile([P, NP * T], BF)
            kT_sb = rpool.tile([P, NP * T], BF)
            v_sb = rpool.tile([P, 16 * VS], BF)
            attn_sb = rpool.tile([P, NP * T], BF)
            wp_sb = rpool.tile([P, NP * C], BF)
            mask_sb = rpool.tile([P, 4 * QC], BF)

            nc.sync.dma_start(mask_sb[:], maskd[:])
            nc.sync.dma_start(
                wp_sb[:].rearrange("p (g c) -> p g c", c=C),
                wp.rearrange("(g p) c -> p g c", p=P),
            )
            nc.vector.memset(
                v_sb[:].rearrange("p (x e) -> p x e", e=65)[:, :, 64:65], 1.0
            )

            # ---------------- Stage 1: q/k/v projections ----------------
            with (
                tc.tile_pool(name="wqkv", bufs=1) as wpool,
                tc.tile_pool(name="xt", bufs=2) as xpool,
                tc.tile_pool(name="ps1", bufs=4, space="PSUM") as ps1,
            ):
                wq_sb = wpool.tile([P, 8 * HC], BF, tag="wq")
                wk_sb = wpool.tile([P, 8 * HC], BF, tag="wk")
                wv_sb = wpool.tile([P, 8 * HC], BF, tag="wv")
                for w_sb, w_d in ((wq_sb, wq), (wk_sb, wk), (wv_sb, wv)):
                    nc.sync.dma_start(
                        w_sb[:].rearrange("p (g h) -> p g h", h=HC),
                        w_d.rearrange("(g p) h -> p g h", p=P),
                    )
                for tb in range(4):
                    ts_ = slice(tb * 512, (tb + 1) * 512)
                    xt = xpool.tile([P, 8 * 512], BF, tag="xt", name=f"xt{tb}")
                    nc.sync.dma_start(
                        xt[:].rearrange("p (g t) -> p g t", t=512),
                        xT.rearrange("(g p) t -> p g t", p=P)[:, :, ts_],
                    )
                    for w_sb, dst in ((wq_sb, qT_sb), (wk_sb, kT_sb)):
                        for hp in range(NP):
                            ps = ps1.tile([P, 512], F32, tag="mm1")
                            for g in range(8):
                                nc.tensor.matmul(
                                    ps[:],
                                    w_sb[:, g * HC + hp * P:][:, :P],
                                    xt[:, g * 512:(g + 1) * 512],
                                    start=(g == 0), stop=(g == 7),
                                )
                            nc.vector.tensor_copy(
                                dst[:, hp * T + tb * 512:][:, :512], ps[:]
                            )
                    for sj in range(4):
                        ps = ps1.tile([P, 512], F32, tag="mm1")
                        for g in range(8):
                            nc.tensor.matmul(
                                ps[:],
                                xt[:, g * 512 + sj * P:][:, :P],
                                wv_sb[:, g * HC:(g + 1) * HC],
                                start=(g == 0), stop=(g == 7),
                            )
                        j = tb * 4 + sj
                        nc.vector.tensor_copy(
                            v_sb[:, j * VS:(j + 1) * VS]
                            .rearrange("p (x e) -> p x e", e=65)[:, :, 0:64],
                            ps[:].rearrange("p (x d) -> p x d", d=64),
                        )

            # ---------------- Stage 2: attention + proj ----------------
            with (
                tc.tile_pool(name="et", bufs=4) as epool,
                tc.tile_pool(name="nrm", bufs=2) as npool,
                tc.tile_pool(name="ost", bufs=2) as opool,
                tc.tile_pool(name="ps_sc", bufs=2, space="PSUM") as ps_sc,
                tc.tile_pool(name="ps_av", bufs=2, space="PSUM") as ps_av,
                tc.tile_pool(name="ps_pj", bufs=2, space="PSUM") as ps_pj,
            ):
                pend = [None]

                def norm(p, qc, avp):
                    rs = npool.tile([1, 2 * QC], F32, tag="rs")
                    nc.vector.tensor_copy(rs[:], avp[64:65, :])
                    avc = npool.tile([64, 2 * QC], F32, tag="avc")
                    nc.vector.tensor_copy(avc[:], avp[0:64, :])
                    rc = npool.tile([1, 2 * QC], F32, tag="rc")
                    nc.vector.reciprocal_approx_fast(rc[:], rs[:])
                    rb = npool.tile([64, 2 * QC], F32, tag="rb")
                    nc.gpsimd.partition_broadcast(rb[:], rc[:])
                    col = p * T + qc * QC
                    nc.vector.tensor_mul(
                        attn_sb[0:64, col:col + QC], avc[:, 0:QC], rb[:, 0:QC]
                    )
                    nc.vector.tensor_mul(
                        attn_sb[64:128, col:col + QC],
                        avc[:, QC:2 * QC], rb[:, QC:2 * QC],
                    )

                def proj(qc):
                    os = opool.tile([P, 2 * C], BF, tag="os")
                    for u in range(2):
                        tt = 2 * qc + u
                        for oc in range(2):
                            pj = ps_pj.tile([P, 512], F32, tag="pj")
                            for p in range(NP):
                                nc.tensor.matmul(
                                    pj[:],
                                    attn_sb[:, p * T + tt * P:][:, :P],
                                    wp_sb[:, p * C + oc * 512:][:, :512],
                                    start=(p == 0), stop=(p == 3),
                                )
                            nc.vector.tensor_copy(
                                os[:, u * C + oc * 512:][:, :512], pj[:]
                            )
                    nc.sync.dma_start(
                        outd[qc * 2 * P:(qc + 1) * 2 * P, :]
                        .rearrange("(u p) c -> p u c", p=P),
                        os[:].rearrange("p (u c) -> p u c", c=C),
                    )

                def emit_av(pv):
                    e_t, avp, p, j0, nk, cb = pv
                    j1 = j0 + 1
                    base0 = j0 * VS + p * 130
                    base1 = j1 * VS + p * 130
                    last = (j1 == nk - 1)
                    # avp is a single PSUM bank: start=True clears has_written
                    # for the WHOLE bank, stop likewise — so exactly one start
                    # (first mm of the bank) and one stop (last mm of the bank).
                    nc.tensor.matmul(avp[:, 0:QC], v_sb[:, base0:base0 + 65],
                                     e_t[:, 0:QC],
                                     start=(j0 == 0), stop=False)
                    nc.tensor.matmul(avp[:, 0:QC], v_sb[:, base1:base1 + 65],
                                     e_t[:, QC:2 * QC],
                                     start=False, stop=False)
                    nc.tensor.matmul(avp[:, QC:2 * QC],
                                     v_sb[:, base0 + 65:base0 + 130],
                                     e_t[:, 2 * QC:3 * QC],
                                     start=False, stop=False)
                    nc.tensor.matmul(avp[:, QC:2 * QC],
                                     v_sb[:, base1 + 65:base1 + 130],
                                     e_t[:, 3 * QC:4 * QC],
                                     start=False, stop=last)
                    if last and cb is not None:
                        cb()

                def c_run(p, qc):
                    nk = 2 * (qc + 1)
                    avp = ps_av.tile([65, 2 * QC], F32, tag="av",
                                     name=f"av{p}_{qc}")
                    qA = qT_sb[0:64, p * T + qc * QC:][:, :QC]
                    qB = qT_sb[64:128, p * T + qc * QC:][:, :QC]

                    def cb(pp=p, qq=qc, aa=avp):
                        norm(pp, qq, aa)
                        if pp == 3:
                            proj(qq)

                    for u in range(qc + 1):
                        j0, j1 = 2 * u, 2 * u + 1
                        kt0 = kT_sb[:, p * T + j0 * P:][:, :P]
                        kt1 = kT_sb[:, p * T + j1 * P:][:, :P]
                        sc = ps_sc.tile([P, 4 * QC], F32, tag="sc")
                        nc.tensor.matmul(sc[:, 0:QC], kt0[0:64, :], qA,
                                         start=True, stop=False,
                                         tile_position=(0, 0))
                        nc.tensor.matmul(sc[:, 2 * QC:3 * QC], kt0[64:128, :],
                                         qB, start=True, stop=False,
                                         tile_position=(64, 0))
                        nc.tensor.matmul(sc[:, QC:2 * QC], kt1[0:64, :], qA,
                                         start=False, stop=True,
                                         tile_position=(0, 0))
                        nc.tensor.matmul(sc[:, 3 * QC:4 * QC], kt1[64:128, :],
                                         qB, start=False, stop=True,
                                         tile_position=(64, 0))
                        e_t = epool.tile([P, 4 * QC], BF, tag="exp")
                        nc.scalar.activation(e_t[:], sc[:], EXP, scale=SCALE)
                        if u == qc:
                            e_m = epool.tile([P, 4 * QC], BF, tag="expm")
                            nc.vector.tensor_mul(e_m[:], e_t[:], mask_sb[:])
                            e_t = e_m
                        if pend[0] is not None:
                            emit_av(pend[0])
                        pend[0] = (e_t, avp, p, j0, nk,
                                   cb if u == qc else None)

                for qc in range(7, -1, -1):
                    for p in range(NP):
                        c_run(p, qc)
                emit_av(pend[0])
    return nc


def _make_mask():
    s = np.arange(P)[:, None]
    t = np.arange(QC)[None, :]
    m0 = (s <= t).astype(np.float32)
    m1 = (s <= t - P).astype(np.float32)
    return np.ascontiguousarray(
        np.concatenate([m0, m1, m0, m1], axis=1).astype(BF16NP)
    )


_CACHE = {}


def _get_nc():
    if "nc" not in _CACHE:
        nc = bacc.Bacc("TRN2", target_bir_lowering=False, debug=False)
        build_kernel(nc)
        nc.compile()
        _CACHE["nc"] = nc
    return _CACHE["nc"]


def make_in_maps(x, wq, wk, wv, w_proj, b_proj):
    x = np.asarray(x, np.float32)
    wq = np.asarray(wq, np.float32)
    wk = np.asarray(wk, np.float32)
    wv = np.asarray(wv, np.float32)
    w_proj = np.asarray(w_proj, np.float32)
    mask = _make_mask()

    halves = []
    for hh in range(2):
        hs = slice(hh * 8, hh * 8 + 8)
        halves.append({
            "wq": np.ascontiguousarray(
                np.transpose(wq[hs], (1, 0, 2)).reshape(C, HC).astype(BF16NP)),
            "wk": np.ascontiguousarray(
                np.transpose(wk[hs], (1, 0, 2)).reshape(C, HC).astype(BF16NP)),
            "wv": np.ascontiguousarray(
                np.transpose(wv[hs], (1, 0, 2)).reshape(C, HC).astype(BF16NP)),
            "wp": np.ascontiguousarray(
                w_proj[hh * HC:(hh + 1) * HC, :].astype(BF16NP)),
        })
    xTs = [np.ascontiguousarray(x[b].T.astype(BF16NP)) for b in range(B)]

    in_maps = []
    for core in range(8):
        b, hh = core // 2, core % 2
        w = halves[hh]
        in_maps.append({
            "xT": xTs[b],
            "wq": w["wq"], "wk": w["wk"], "wv": w["wv"], "wp": w["wp"],
            "maskd": mask,
        })
    return in_maps


def assemble(results, b_proj):
    bias = np.asarray(b_proj, np.float32)
    full = np.empty((B, T, C), np.float32)
    for b in range(B):
        p0 = np.asarray(results[2 * b]["out"]).astype(np.float32)
        p1 = np.asarray(results[2 * b + 1]["out"]).astype(np.float32)
        full[b] = p0 + p1 + bias[None, :]
    return full


def kernel(x, wq, wk, wv, w_proj, b_proj, _trace=False, _tmpdir=None):
    in_maps = make_in_maps(x, wq, wk, wv, w_proj, b_proj)
    nc = _get_nc()
    res = run_bass_kernel_spmd(
        nc, in_maps, core_ids=list(range(8)), trace=_trace, tmpdir=_tmpdir
    )
    if _trace:
        _CACHE["last_result"] = res
    return assemble(res.results, b_proj)


# revision 7
# speedup vs baseline: 2.2101x; 1.4169x over previous
"""Multi-head causal attention (B=4,T=2048,C=1024,H=16,D=64) on 8 TRN2 NeuronCores.

Sharding: batch x head-half tensor parallel. Core c handles batch b=c//2 and
heads [8*(c%2), 8*(c%2)+8) over ALL 2048 queries. Each core computes its own
Q/K/V projections (no duplicated work, no cross-core traffic), causal
attention for its 8 heads, and a partial output projection
attn_half @ w_proj[half_rows]. The host unshards by summing the two partials
per batch and adding the bias (the all-reduce of the TP sharding, done at
gather time). Causal load is uniform per core by construction.

Per-core program (bf16 matmuls, fp32 PSUM). The scalar engine's exp stream is
the scarce resource (~190us), so the program is a single software-pipelined
stream that keeps it fed from ~15% in:
  1. Q projections for all 2048 queries up front (PE-dense prologue).
  2. K/V projections are chopped into per-matmul "filler" generators,
     interleaved a couple of matmuls per attention unit so the PE stays busy
     while the scalar engine paces the exp stream; a query chunk's attention
     starts as soon as its causal key prefix is projected.
  3. Attention unit (head-pair, 256-query chunk, PAIR of 128-key tiles):
     scores into a 2-bank PSUM tile (2 heads via PE row groups -> separate
     banks; one start/stop per bank) -> exp on ACT (scale folded) -> 0/1
     causal mask on DVE (diagonal unit only) -> wei @ [v|1] accumulated in
     PSUM ([65,256]/head, row 64 = sumexp). AV emission is delayed two units
     so the PE never waits on ACT. Normalization: DVE fast reciprocal +
     GPSIMD partition broadcast.
  4. The partial projection for a query chunk is queued as another filler
     generator once its last head-pair is normalized; per-chunk output DMA.
All dense-GEMM PSUM tiles (Q/K/V/proj) share one 2-buffer [128,512] ring so
the whole program fits the 8 PSUM banks alongside double-buffered score and
AV accumulators.
"""

from collections import deque

import numpy as np
import ml_dtypes

import concourse.bass as bass
import concourse.tile as tile
from concourse import bacc, library_config, mybir
from concourse.bass_utils import run_bass_kernel_spmd

B, T, C = 4, 2048, 1024
H, D = 16, 64
P = 128           # key tile size
QC = 256          # query chunk size
HC = 512          # head channels per core (8 heads x 64)
NP = 4            # head pairs per core
BF = mybir.dt.bfloat16
F32 = mybir.dt.float32
BF16NP = ml_dtypes.bfloat16
EXP = mybir.ActivationFunctionType.Exp
SCALE = float(C) ** -0.5
VS = 2 * NP * 65  # 520: per key-tile v row: 4 pairs x (2 heads x 65)


def build_kernel(nc: bass.Bass):
    xT = nc.dram_tensor("xT", [C, T], BF, kind="ExternalInput").ap()
    wq = nc.dram_tensor("wq", [C, HC], BF, kind="ExternalInput").ap()
    wk = nc.dram_tensor("wk", [C, HC], BF, kind="ExternalInput").ap()
    wv = nc.dram_tensor("wv", [C, HC], BF, kind="ExternalInput").ap()
    wp = nc.dram_tensor("wp", [HC, C], BF, kind="ExternalInput").ap()
    maskd = nc.dram_tensor("maskd", [P, 4 * QC], BF, kind="ExternalInput").ap()
    outd = nc.dram_tensor("out", [T, C], BF, kind="ExternalOutput").ap()

    with tile.TileContext(nc) as tc:
        nc.gpsimd.load_library(library_config.attn)
        with (
            tc.tile_pool(name="res", bufs=1) as rpool,
            tc.tile_pool(name="wqkv", bufs=1) as wpool,
            tc.tile_pool(name="xt", bufs=4) as xpool,
            tc.tile_pool(name="et", bufs=4) as epool,
            tc.tile_pool(name="nrm", bufs=2) as npool,
            tc.tile_pool(name="ost", bufs=2) as opool,
            tc.tile_pool(name="ps_gp", bufs=2, space="PSUM") as ps_gp,
            tc.tile_pool(name="ps_sc", bufs=2, space="PSUM") as ps_sc,
            tc.tile_pool(name="ps_av", bufs=2, space="PSUM") as ps_av,
        ):
            qT_sb = rpool.tile([P, NP * T], BF)
            kT_sb = rpool.tile([P, NP * T], BF)
            v_sb = rpool.tile([P, 16 * VS], BF)
            attn_sb = rpool.tile([P, NP * T], BF)
            wp_sb = rpool.tile([P, NP * C], BF)
            mask_sb = rpool.tile([P, 4 * QC], BF)
            wq_sb = wpool.tile([P, 8 * HC], BF, tag="wq")
            wk_sb = wpool.tile([P, 8 * HC], BF, tag="wk")
            wv_sb = wpool.tile([P, 8 * HC], BF, tag="wv")

            # DMA order = first-needed first: xt0+wq gate the first matmul.
            xts = [
                xpool.tile([P, 8 * 512], BF, tag="xt", name=f"xt{tb}")
                for tb in range(4)
            ]
            nc.sync.dma_start(
                xts[0][:].rearrange("p (g t) -> p g t", t=512),
                xT.rearrange("(g p) t -> p g t", p=P)[:, :, 0:512],
            )
            nc.sync.dma_start(
                wq_sb[:].rearrange("p (g h) -> p g h", h=HC),
                wq.rearrange("(g p) h -> p g h", p=P),
            )
            for tb in range(1, 4):
                nc.sync.dma_start(
                    xts[tb][:].rearrange("p (g t) -> p g t", t=512),
                    xT.rearrange("(g p) t -> p g t", p=P)
                    [:, :, tb * 512:(tb + 1) * 512],
                )
            for w_sb, w_d in ((wk_sb, wk), (wv_sb, wv)):
                nc.sync.dma_start(
                    w_sb[:].rearrange("p (g h) -> p g h", h=HC),
                    w_d.rearrange("(g p) h -> p g h", p=P),
                )
            nc.sync.dma_start(mask_sb[:], maskd[:])
            nc.sync.dma_start(
                wp_sb[:].rearrange("p (g c) -> p g c", c=C),
                wp.rearrange("(g p) c -> p g c", p=P),
            )
            nc.vector.memset(
                v_sb[:].rearrange("p (x e) -> p x e", e=65)[:, :, 64:65], 1.0
            )

            # ---------------- Q prologue ----------------
            for tb in range(4):
                for hp in range(NP):
                    ps = ps_gp.tile([P, 512], F32, tag="gp", name=f"q{tb}{hp}")
                    for g in range(8):
                        nc.tensor.matmul(
                            ps[:],
                            wq_sb[:, g * HC + hp * P:][:, :P],
                            xts[tb][:, g * 512:(g + 1) * 512],
                            start=(g == 0), stop=(g == 7),
                        )
                    nc.vector.tensor_copy(
                        qT_sb[:, hp * T + tb * 512:][:, :512], ps[:]
                    )

            # ---------------- filler machinery ----------------
            state = {"tb_done": -1}
            fill = []

            def kv_gen():
                for tb in range(4):
                    for hp in range(NP):
                        ps = ps_gp.tile([P, 512], F32, tag="gp",
                                        name=f"k{tb}{hp}")
                        for g in range(8):
                            nc.tensor.matmul(
                                ps[:],
                                wk_sb[:, g * HC + hp * P:][:, :P],
                                xts[tb][:, g * 512:(g + 1) * 512],
                                start=(g == 0), stop=(g == 7),
                            )
                            yield
                        nc.vector.tensor_copy(
                            kT_sb[:, hp * T + tb * 512:][:, :512], ps[:]
                        )
                        yield
                    for sj in range(4):
                        ps = ps_gp.tile([P, 512], F32, tag="gp",
                                        name=f"v{tb}{sj}")
                        for g in range(8):
                            nc.tensor.matmul(
                                ps[:],
                                xts[tb][:, g * 512 + sj * P:][:, :P],
                                wv_sb[:, g * HC:(g + 1) * HC],
                                start=(g == 0), stop=(g == 7),
                            )
                            yield
                        j = tb * 4 + sj
                        nc.vector.tensor_copy(
                            v_sb[:, j * VS:(j + 1) * VS]
                            .rearrange("p (x e) -> p x e", e=65)[:, :, 0:64],
                            ps[:].rearrange("p (x d) -> p x d", d=64),
                        )
                        yield
                    state["tb_done"] = tb

            def proj_gen(qc):
                os = opool.tile([P, 2 * C], BF, tag="os", name=f"os{qc}")
                for u2 in range(2):
                    tt = 2 * qc + u2
                    for oc in range(2):
                        pj = ps_gp.tile([P, 512], F32, tag="gp",
                                        name=f"pj{qc}{u2}{oc}")
                        for p in range(NP):
                            nc.tensor.matmul(
                                pj[:],
                                attn_sb[:, p * T + tt * P:][:, :P],
                                wp_sb[:, p * C + oc * 512:][:, :512],
                                start=(p == 0), stop=(p == 3),
                            )
                            yield
                        nc.vector.tensor_copy(
                            os[:, u2 * C + oc * 512:][:, :512], pj[:]
                        )
                        yield
                nc.sync.dma_start(
                    outd[qc * 2 * P:(qc + 1) * 2 * P, :]
                    .rearrange("(u p) c -> p u c", p=P),
                    os[:].rearrange("p (u c) -> p u c", c=C),
                )

            def pop_fill(n=1):
                for _ in range(n):
                    while fill:
                        try:
                            next(fill[0])
                            break
                        except StopIteration:
                            fill.pop(0)
                    else:
                        return

            def drain_until_tb(tb):
                while state["tb_done"] < tb:
                    try:
                        next(fill[0])
                    except StopIteration:
                        fill.pop(0)

            # ---------------- attention ----------------
            pend = deque()

            def norm(p, qc, avp):
                rs = npool.tile([1, 2 * QC], F32, tag="rs")
                nc.vector.tensor_copy(rs[:], avp[64:65, :])
                avc = npool.tile([64, 2 * QC], F32, tag="avc")
                nc.vector.tensor_copy(avc[:], avp[0:64, :])
                rc = npool.tile([1, 2 * QC], F32, tag="rc")
                nc.vector.reciprocal_approx_fast(rc[:], rs[:])
                rb = npool.tile([64, 2 * QC], F32, tag="rb")
                nc.gpsimd.partition_broadcast(rb[:], rc[:])
                col = p * T + qc * QC
                nc.vector.tensor_mul(
                    attn_sb[0:64, col:col + QC], avc[:, 0:QC], rb[:, 0:QC]
                )
                nc.vector.tensor_mul(
                    attn_sb[64:128, col:col + QC],
                    avc[:, QC:2 * QC], rb[:, QC:2 * QC],
                )

            def emit_av(pv):
                e_t, avp, p, j0, nk, cb = pv
                j1 = j0 + 1
                base0 = j0 * VS + p * 130
                base1 = j1 * VS + p * 130
                last = (j1 == nk - 1)
                # avp is a single PSUM bank: exactly one start (first mm of
                # the bank) and one stop (last mm of the bank).
                nc.tensor.matmul(avp[:, 0:QC], v_sb[:, base0:base0 + 65],
                                 e_t[:, 0:QC],
                                 start=(j0 == 0), stop=False)
                nc.tensor.matmul(avp[:, 0:QC], v_sb[:, base1:base1 + 65],
                                 e_t[:, QC:2 * QC],
                                 start=False, stop=False)
                nc.tensor.matmul(avp[:, QC:2 * QC],
                                 v_sb[:, base0 + 65:base0 + 130],
                                 e_t[:, 2 * QC:3 * QC],
                                 start=False, stop=False)
                nc.tensor.matmul(avp[:, QC:2 * QC],
                                 v_sb[:, base1 + 65:base1 + 130],
                                 e_t[:, 3 * QC:4 * QC],
                                 start=False, stop=last)
                if last and cb is not None:
                    cb()

            def c_run(p, qc):
                nk = 2 * (qc + 1)
                avp = ps_av.tile([65, 2 * QC], F32, tag="av",
                                 name=f"av{p}_{qc}")
                qA = qT_sb[0:64, p * T + qc * QC:][:, :QC]
                qB = qT_sb[64:128, p * T + qc * QC:][:, :QC]

                def cb(pp=p, qq=qc, aa=avp):
                    norm(pp, qq, aa)
                    if pp == 3:
                        fill.append(proj_gen(qq))

                for u in range(qc + 1):
                    j0, j1 = 2 * u, 2 * u + 1
                    kt0 = kT_sb[:, p * T + j0 * P:][:, :P]
                    kt1 = kT_sb[:, p * T + j1 * P:][:, :P]
                    # sc spans 2 PSUM banks: head A cols [0:2QC] (bank 0),
                    # head B cols [2QC:4QC] (bank 1) -> concurrent row-group
                    # matmuls land in different banks; one start/stop per bank.
                    sc = ps_sc.tile([P, 4 * QC], F32, tag="sc")
                    nc.tensor.matmul(sc[:, 0:QC], kt0[0:64, :], qA,
                                     start=True, stop=False,
                                     tile_position=(0, 0))
                    nc.tensor.matmul(sc[:, 2 * QC:3 * QC], kt0[64:128, :],
                                     qB, start=True, stop=False,
                                     tile_position=(64, 0))
                    nc.tensor.matmul(sc[:, QC:2 * QC], kt1[0:64, :], qA,
                                     start=False, stop=True,
                                     tile_position=(0, 0))
                    nc.tensor.matmul(sc[:, 3 * QC:4 * QC], kt1[64:128, :],
                                     qB, start=False, stop=True,
                                     tile_position=(64, 0))
                    e_t = epool.tile([P, 4 * QC], BF, tag="exp")
                    nc.scalar.activation(e_t[:], sc[:], EXP, scale=SCALE)
                    if u == qc:
                        e_m = epool.tile([P, 4 * QC], BF, tag="expm", bufs=2)
                        nc.vector.tensor_mul(e_m[:], e_t[:], mask_sb[:])
                        e_t = e_m
                    if len(pend) >= 2:
                        emit_av(pend.popleft())
                    pend.append((e_t, avp, p, j0, nk,
                                 cb if u == qc else None))
                    pop_fill(2)

            fill.append(kv_gen())
            for qc in range(8):
                drain_until_tb(qc // 2)
                for p in range(NP):
                    c_run(p, qc)
            while pend:
                emit_av(pend.popleft())
            pop_fill(10 ** 6)
    return nc


def _make_mask():
    s = np.arange(P)[:, None]
    t = np.arange(QC)[None, :]
    m0 = (s <= t).astype(np.float32)
    m1 = (s <= t - P).astype(np.float32)
    return np.ascontiguousarray(
        np.concatenate([m0, m1, m0, m1], axis=1).astype(BF16NP)
    )


_CACHE = {}


def _get_nc():
    if "nc" not in _CACHE:
        nc = bacc.Bacc("TRN2", target_bir_lowering=False, debug=False)
        build_kernel(nc)
        nc.compile()
        _CACHE["nc"] = nc
    return _CACHE["nc"]


def make_in_maps(x, wq, wk, wv, w_proj, b_proj):
    x = np.asarray(x, np.float32)
    wq = np.asarray(wq, np.float32)
    wk = np.asarray(wk, np.float32)
    wv = np.asarray(wv, np.float32)
    w_proj = np.asarray(w_proj, np.float32)
    mask = _make_mask()

    halves = []
    for hh in range(2):
        hs = slice(hh * 8, hh * 8 + 8)
        halves.append({
            "wq": np.ascontiguousarray(
                np.transpose(wq[hs], (1, 0, 2)).reshape(C, HC).astype(BF16NP)),
            "wk": np.ascontiguousarray(
                np.transpose(wk[hs], (1, 0, 2)).reshape(C, HC).astype(BF16NP)),
            "wv": np.ascontiguousarray(
                np.transpose(wv[hs], (1, 0, 2)).reshape(C, HC).astype(BF16NP)),
            "wp": np.ascontiguousarray(
                w_proj[hh * HC:(hh + 1) * HC, :].astype(BF16NP)),
        })
    xTs = [np.ascontiguousarray(x[b].T.astype(BF16NP)) for b in range(B)]

    in_maps = []
    for core in range(8):
        b, hh = core // 2, core % 2
        w = halves[hh]
        in_maps.append({
            "xT": xTs[b],
            "wq": w["wq"], "wk": w["wk"], "wv": w["wv"], "wp": w["wp"],
            "maskd": mask,
        })
    return in_maps


def assemble(results, b_proj):
    bias = np.asarray(b_proj, np.float32)
    full = np.empty((B, T, C), np.float32)
    for b in range(B):
        p0 = np.asarray(results[2 * b]["out"]).astype(np.float32)
        p1 = np.asarray(results[2 * b + 1]["out"]).astype(np.float32)
        full[b] = p0 + p1 + bias[None, :]
    return full


def kernel(x, wq, wk, wv, w_proj, b_proj, _trace=False, _tmpdir=None):
    in_maps = make_in_maps(x, wq, wk, wv, w_proj, b_proj)
    nc = _get_nc()
    res = run_bass_kernel_spmd(
        nc, in_maps, core_ids=list(range(8)), trace=_trace, tmpdir=_tmpdir
    )
    if _trace:
        _CACHE["last_result"] = res
    return assemble(res.results, b_proj)


# revision 8
# speedup vs baseline: 2.2365x; 1.0119x over previous
"""Multi-head causal attention (B=4,T=2048,C=1024,H=16,D=64) on 8 TRN2 NeuronCores.

Sharding: batch x head-half tensor parallel. Core c handles batch b=c//2 and
heads [8*(c%2), 8*(c%2)+8) over ALL 2048 queries. Each core computes its own
Q/K/V projections (no duplicated work, no cross-core traffic), causal
attention for its 8 heads, and a partial output projection
attn_half @ w_proj[half_rows]. The host unshards by summing the two partials
per batch and adding the bias (the all-reduce of the TP sharding, done at
gather time). Causal load is uniform per core by construction.

Per-core program (bf16 matmuls, fp32 PSUM). The scalar engine's exp stream is
the scarce resource (~190us), so the program is a single software-pipelined
stream that keeps it fed from ~15% in:
  1. Q projections for all 2048 queries up front (PE-dense prologue).
  2. K/V projections are chopped into per-matmul "filler" generators,
     interleaved a couple of matmuls per attention unit so the PE stays busy
     while the scalar engine paces the exp stream; a query chunk's attention
     starts as soon as its causal key prefix is projected.
  3. Attention unit (head-pair, 256-query chunk, PAIR of 128-key tiles):
     scores into a 2-bank PSUM tile (2 heads via PE row groups -> separate
     banks; one start/stop per bank) -> exp on ACT (scale folded) -> 0/1
     causal mask on DVE (diagonal unit only) -> wei @ [v|1] accumulated in
     PSUM ([65,256]/head, row 64 = sumexp). AV emission is delayed two units
     so the PE never waits on ACT. Normalization: DVE fast reciprocal +
     GPSIMD partition broadcast.
  4. The partial projection for a query chunk is queued as another filler
     generator once its last head-pair is normalized; per-chunk output DMA.
All dense-GEMM PSUM tiles (Q/K/V/proj) share one 2-buffer [128,512] ring so
the whole program fits the 8 PSUM banks alongside double-buffered score and
AV accumulators.
"""

from collections import deque

import numpy as np
import ml_dtypes

import concourse.bass as bass
import concourse.tile as tile
from concourse import bacc, library_config, mybir
from concourse.bass_utils import run_bass_kernel_spmd

B, T, C = 4, 2048, 1024
H, D = 16, 64
P = 128           # key tile size
QC = 256          # query chunk size
HC = 512          # head channels per core (8 heads x 64)
NP = 4            # head pairs per core
BF = mybir.dt.bfloat16
F32 = mybir.dt.float32
BF16NP = ml_dtypes.bfloat16
EXP = mybir.ActivationFunctionType.Exp
SCALE = float(C) ** -0.5
VS = 2 * NP * 65  # 520: per key-tile v row: 4 pairs x (2 heads x 65)


def build_kernel(nc: bass.Bass):
    xT = nc.dram_tensor("xT", [C, T], BF, kind="ExternalInput").ap()
    wq = nc.dram_tensor("wq", [C, HC], BF, kind="ExternalInput").ap()
    wk = nc.dram_tensor("wk", [C, HC], BF, kind="ExternalInput").ap()
    wv = nc.dram_tensor("wv", [C, HC], BF, kind="ExternalInput").ap()
    wp = nc.dram_tensor("wp", [HC, C], BF, kind="ExternalInput").ap()
    maskd = nc.dram_tensor("maskd", [P, 4 * QC], BF, kind="ExternalInput").ap()
    outd = nc.dram_tensor("out", [T, C], BF, kind="ExternalOutput").ap()

    with tile.TileContext(nc) as tc:
        nc.gpsimd.load_library(library_config.attn)
        with (
            tc.tile_pool(name="res", bufs=1) as rpool,
            tc.tile_pool(name="wqkv", bufs=1) as wpool,
            tc.tile_pool(name="xt", bufs=4) as xpool,
            tc.tile_pool(name="et", bufs=4) as epool,
            tc.tile_pool(name="nrm", bufs=2) as npool,
            tc.tile_pool(name="ost", bufs=2) as opool,
            tc.tile_pool(name="ps_gp", bufs=2, space="PSUM") as ps_gp,
            tc.tile_pool(name="ps_sc", bufs=2, space="PSUM") as ps_sc,
            tc.tile_pool(name="ps_av", bufs=2, space="PSUM") as ps_av,
        ):
            qT_sb = rpool.tile([P, NP * T], BF)
            kT_sb = rpool.tile([P, NP * T], BF)
            v_sb = rpool.tile([P, 16 * VS], BF)
            attn_sb = rpool.tile([P, NP * T], BF)
            wp_sb = rpool.tile([P, NP * C], BF)
            mask_sb = rpool.tile([P, 4 * QC], BF)
            wq_sb = wpool.tile([P, 8 * HC], BF, tag="wq")
            wk_sb = wpool.tile([P, 8 * HC], BF, tag="wk")
            wv_sb = wpool.tile([P, 8 * HC], BF, tag="wv")

            # DMA order = first-needed first: xt0+wq gate the first matmul.
            xts = [
                xpool.tile([P, 8 * 512], BF, tag="xt", name=f"xt{tb}")
                for tb in range(4)
            ]
            xt0_dst = xts[0][:].rearrange("p (g t) -> p g t", t=512)
            xt0_src = xT.rearrange("(g p) t -> p g t", p=P)[:, :, 0:512]
            wq_dst = wq_sb[:].rearrange("p (g h) -> p g h", h=HC)
            wq_src = wq.rearrange("(g p) h -> p g h", p=P)
            nc.sync.dma_start(xt0_dst[:, 0:4], xt0_src[:, 0:4])
            nc.sync.dma_start(wq_dst[:, :, 0:P], wq_src[:, :, 0:P])
            nc.sync.dma_start(xt0_dst[:, 4:8], xt0_src[:, 4:8])
            for hp in range(1, 4):
                nc.sync.dma_start(
                    wq_dst[:, :, hp * P:(hp + 1) * P],
                    wq_src[:, :, hp * P:(hp + 1) * P],
                )
            for tb in range(1, 4):
                nc.sync.dma_start(
                    xts[tb][:].rearrange("p (g t) -> p g t", t=512),
                    xT.rearrange("(g p) t -> p g t", p=P)
                    [:, :, tb * 512:(tb + 1) * 512],
                )
            for w_sb, w_d in ((wk_sb, wk), (wv_sb, wv)):
                nc.sync.dma_start(
                    w_sb[:].rearrange("p (g h) -> p g h", h=HC),
                    w_d.rearrange("(g p) h -> p g h", p=P),
                )
            nc.sync.dma_start(mask_sb[:], maskd[:])
            nc.sync.dma_start(
                wp_sb[:].rearrange("p (g c) -> p g c", c=C),
                wp.rearrange("(g p) c -> p g c", p=P),
            )
            nc.vector.memset(
                v_sb[:].rearrange("p (x e) -> p x e", e=65)[:, :, 64:65], 1.0
            )

            # ---------------- Q prologue ----------------
            for tb in range(4):
                for hp in range(NP):
                    ps = ps_gp.tile([P, 512], F32, tag="gp", name=f"q{tb}{hp}")
                    for g in range(8):
                        nc.tensor.matmul(
                            ps[:],
                            wq_sb[:, g * HC + hp * P:][:, :P],
                            xts[tb][:, g * 512:(g + 1) * 512],
                            start=(g == 0), stop=(g == 7),
                        )
                    nc.vector.tensor_copy(
                        qT_sb[:, hp * T + tb * 512:][:, :512], ps[:]
                    )

            # ---------------- filler machinery ----------------
            state = {"tb_done": -1}
            fill = []

            def kv_gen():
                for tb in range(4):
                    for hp in range(NP):
                        ps = ps_gp.tile([P, 512], F32, tag="gp",
                                        name=f"k{tb}{hp}")
                        for g in range(8):
                            nc.tensor.matmul(
                                ps[:],
                                wk_sb[:, g * HC + hp * P:][:, :P],
                                xts[tb][:, g * 512:(g + 1) * 512],
                                start=(g == 0), stop=(g == 7),
                            )
                            yield
                        nc.vector.tensor_copy(
                            kT_sb[:, hp * T + tb * 512:][:, :512], ps[:]
                        )
                        yield
                    for sj in range(4):
                        ps = ps_gp.tile([P, 512], F32, tag="gp",
                                        name=f"v{tb}{sj}")
                        for g in range(8):
                            nc.tensor.matmul(
                                ps[:],
                                xts[tb][:, g * 512 + sj * P:][:, :P],
                                wv_sb[:, g * HC:(g + 1) * HC],
                                start=(g == 0), stop=(g == 7),
                            )
                            yield
                        j = tb * 4 + sj
                        nc.vector.tensor_copy(
                            v_sb[:, j * VS:(j + 1) * VS]
                            .rearrange("p (x e) -> p x e", e=65)[:, :, 0:64],
                            ps[:].rearrange("p (x d) -> p x d", d=64),
                        )
                        yield
                    state["tb_done"] = tb

            def proj_gen(qc):
                os = opool.tile([P, 2 * C], BF, tag="os", name=f"os{qc}")
                for u2 in range(2):
                    tt = 2 * qc + u2
                    for oc in range(2):
                        pj = ps_gp.tile([P, 512], F32, tag="gp",
                                        name=f"pj{qc}{u2}{oc}")
                        for p in range(NP):
                            nc.tensor.matmul(
                                pj[:],
                                attn_sb[:, p * T + tt * P:][:, :P],
                                wp_sb[:, p * C + oc * 512:][:, :512],
                                start=(p == 0), stop=(p == 3),
                            )
                            yield
                        nc.vector.tensor_copy(
                            os[:, u2 * C + oc * 512:][:, :512], pj[:]
                        )
                        yield
                nc.sync.dma_start(
                    outd[qc * 2 * P:(qc + 1) * 2 * P, :]
                    .rearrange("(u p) c -> p u c", p=P),
                    os[:].rearrange("p (u c) -> p u c", c=C),
                )

            def pop_fill(n=1):
                for _ in range(n):
                    while fill:
                        try:
                            next(fill[0])
                            break
                        except StopIteration:
                            fill.pop(0)
                    else:
                        return

            def drain_until_tb(tb):
                while state["tb_done"] < tb:
                    try:
                        next(fill[0])
                    except StopIteration:
                        fill.pop(0)

            # ---------------- attention ----------------
            pend = deque()

            def norm(p, qc, avp):
                rs = npool.tile([1, 2 * QC], F32, tag="rs")
                nc.vector.tensor_copy(rs[:], avp[64:65, :])
                avc = npool.tile([64, 2 * QC], F32, tag="avc")
                nc.vector.tensor_copy(avc[:], avp[0:64, :])
                rc = npool.tile([1, 2 * QC], F32, tag="rc")
                nc.vector.reciprocal_approx_fast(rc[:], rs[:])
                rb = npool.tile([64, 2 * QC], F32, tag="rb")
                nc.gpsimd.partition_broadcast(rb[:], rc[:])
                col = p * T + qc * QC
                nc.vector.tensor_mul(
                    attn_sb[0:64, col:col + QC], avc[:, 0:QC], rb[:, 0:QC]
                )
                nc.vector.tensor_mul(
                    attn_sb[64:128, col:col + QC],
                    avc[:, QC:2 * QC], rb[:, QC:2 * QC],
                )

            def emit_av(pv):
                e_t, avp, p, j0, nk, cb = pv
                j1 = j0 + 1
                base0 = j0 * VS + p * 130
                base1 = j1 * VS + p * 130
                last = (j1 == nk - 1)
                # avp is a single PSUM bank: exactly one start (first mm of
                # the bank) and one stop (last mm of the bank).
                nc.tensor.matmul(avp[:, 0:QC], v_sb[:, base0:base0 + 65],
                                 e_t[:, 0:QC],
                                 start=(j0 == 0), stop=False)
                nc.tensor.matmul(avp[:, 0:QC], v_sb[:, base1:base1 + 65],
                                 e_t[:, QC:2 * QC],
                                 start=False, stop=False)
                nc.tensor.matmul(avp[:, QC:2 * QC],
                                 v_sb[:, base0 + 65:base0 + 130],
                                 e_t[:, 2 * QC:3 * QC],
                                 start=False, stop=False)
                nc.tensor.matmul(avp[:, QC:2 * QC],
                                 v_sb[:, base1 + 65:base1 + 130],
                                 e_t[:, 3 * QC:4 * QC],
                                 start=False, stop=last)
                if last and cb is not None:
                    cb()

            def c_run(p, qc):
                nk = 2 * (qc + 1)
                avp = ps_av.tile([65, 2 * QC], F32, tag="av",
                                 name=f"av{p}_{qc}")
                qA = qT_sb[0:64, p * T + qc * QC:][:, :QC]
                qB = qT_sb[64:128, p * T + qc * QC:][:, :QC]

                def cb(pp=p, qq=qc, aa=avp):
                    norm(pp, qq, aa)
                    if pp == 3:
                        fill.append(proj_gen(qq))

                for u in range(qc + 1):
                    j0, j1 = 2 * u, 2 * u + 1
                    kt0 = kT_sb[:, p * T + j0 * P:][:, :P]
                    kt1 = kT_sb[:, p * T + j1 * P:][:, :P]
                    # sc spans 2 PSUM banks: head A cols [0:2QC] (bank 0),
                    # head B cols [2QC:4QC] (bank 1) -> concurrent row-group
                    # matmuls land in different banks; one start/stop per bank.
                    sc = ps_sc.tile([P, 4 * QC], F32, tag="sc")
                    nc.tensor.matmul(sc[:, 0:QC], kt0[0:64, :], qA,
                                     start=True, stop=False,
                                     tile_position=(0, 0))
                    nc.tensor.matmul(sc[:, 2 * QC:3 * QC], kt0[64:128, :],
                                     qB, start=True, stop=False,
                                     tile_position=(64, 0))
                    nc.tensor.matmul(sc[:, QC:2 * QC], kt1[0:64, :], qA,
                                     start=False, stop=True,
                                     tile_position=(0, 0))
                    nc.tensor.matmul(sc[:, 3 * QC:4 * QC], kt1[64:128, :],
                                     qB, start=False, stop=True,
                                     tile_position=(64, 0))
                    e_t = epool.tile([P, 4 * QC], BF, tag="exp")
                    nc.scalar.activation(e_t[:], sc[:], EXP, scale=SCALE)
                    if u == qc:
                        e_m = epool.tile([P, 4 * QC], BF, tag="expm", bufs=2)
                        nc.vector.tensor_mul(e_m[:], e_t[:], mask_sb[:])
                        e_t = e_m
                    if len(pend) >= 2:
                        emit_av(pend.popleft())
                    pend.append((e_t, avp, p, j0, nk,
                                 cb if u == qc else None))
                    pop_fill(2)

            fill.append(kv_gen())
            for qc in range(8):
                drain_until_tb(qc // 2)
                for p in range(NP):
                    c_run(p, qc)
            while pend:
                emit_av(pend.popleft())
            pop_fill(10 ** 6)
    return nc


def _make_mask():
    s = np.arange(P)[:, None]
    t = np.arange(QC)[None, :]
    m0 = (s <= t).astype(np.float32)
    m1 = (s <= t - P).astype(np.float32)
    return np.ascontiguousarray(
        np.concatenate([m0, m1, m0, m1], axis=1).astype(BF16NP)
    )


_CACHE = {}


def _get_nc():
    if "nc" not in _CACHE:
        nc = bacc.Bacc("TRN2", target_bir_lowering=False, debug=False)
        build_kernel(nc)
        nc.compile()
        _CACHE["nc"] = nc
    return _CACHE["nc"]


def make_in_maps(x, wq, wk, wv, w_proj, b_proj):
    x = np.asarray(x, np.float32)
    wq = np.asarray(wq, np.float32)
    wk = np.asarray(wk, np.float32)
    wv = np.asarray(wv, np.float32)
    w_proj = np.asarray(w_proj, np.float32)
    mask = _make_mask()

    halves = []
    for hh in range(2):
        hs = slice(hh * 8, hh * 8 + 8)
        halves.append({
            "wq": np.ascontiguousarray(
                np.transpose(wq[hs], (1, 0, 2)).reshape(C, HC).astype(BF16NP)),
            "wk": np.ascontiguousarray(
                np.transpose(wk[hs], (1, 0, 2)).reshape(C, HC).astype(BF16NP)),
            "wv": np.ascontiguousarray(
                np.transpose(wv[hs], (1, 0, 2)).reshape(C, HC).astype(BF16NP)),
            "wp": np.ascontiguousarray(
                w_proj[hh * HC:(hh + 1) * HC, :].astype(BF16NP)),
        })
    xTs = [np.ascontiguousarray(x[b].T.astype(BF16NP)) for b in range(B)]

    in_maps = []
    for core in range(8):
        b, hh = core // 2, core % 2
        w = halves[hh]
        in_maps.append({
            "xT": xTs[b],
            "wq": w["wq"], "wk": w["wk"], "wv": w["wv"], "wp": w["wp"],
            "maskd": mask,
        })
    return in_maps


def assemble(results, b_proj):
    bias = np.asarray(b_proj, np.float32)
    full = np.empty((B, T, C), np.float32)
    for b in range(B):
        p0 = np.asarray(results[2 * b]["out"]).astype(np.float32)
        p1 = np.asarray(results[2 * b + 1]["out"]).astype(np.float32)
        full[b] = p0 + p1 + bias[None, :]
    return full


def kernel(x, wq, wk, wv, w_proj, b_proj, _trace=False, _tmpdir=None):
    in_maps = make_in_maps(x, wq, wk, wv, w_proj, b_proj)
    nc = _get_nc()
    res = run_bass_kernel_spmd(
        nc, in_maps, core_ids=list(range(8)), trace=_trace, tmpdir=_tmpdir
    )
    if _trace:
        _CACHE["last_result"] = res
    return assemble(res.results, b_proj)


# revision 11
# speedup vs baseline: 2.2418x; 1.0024x over previous
"""Multi-head causal attention (B=4,T=2048,C=1024,H=16,D=64) on 8 TRN2 NeuronCores.

Sharding: batch x head-half tensor parallel. Core c handles batch b=c//2 and
heads [8*(c%2), 8*(c%2)+8) over ALL 2048 queries. Each core computes its own
Q/K/V projections (no duplicated work, no cross-core traffic), causal
attention for its 8 heads, and a partial output projection
attn_half @ w_proj[half_rows]. The host unshards by summing the two partials
per batch and adding the bias (the all-reduce of the TP sharding, done at
gather time). Causal load is uniform per core by construction.

Per-core program (bf16 matmuls, fp32 PSUM). The scalar engine's exp stream is
the scarce resource (~190us), so the program is a single software-pipelined
stream that keeps it fed from ~15% in:
  1. Q projections for all 2048 queries up front (PE-dense prologue).
  2. K/V projections are chopped into per-matmul "filler" generators,
     interleaved a couple of matmuls per attention unit so the PE stays busy
     while the scalar engine paces the exp stream; a query chunk's attention
     starts as soon as its causal key prefix is projected.
  3. Attention unit (head-pair, 256-query chunk, PAIR of 128-key tiles):
     scores into a 2-bank PSUM tile (2 heads via PE row groups -> separate
     banks; one start/stop per bank) -> exp on ACT (scale folded) -> 0/1
     causal mask on DVE (diagonal unit only) -> wei @ [v|1] accumulated in
     PSUM ([65,256]/head, row 64 = sumexp). AV emission is delayed two units
     so the PE never waits on ACT. Normalization: DVE fast reciprocal +
     GPSIMD partition broadcast.
  4. The partial projection for a query chunk is queued as another filler
     generator once its last head-pair is normalized; per-chunk output DMA.
All dense-GEMM PSUM tiles (Q/K/V/proj) share one 2-buffer [128,512] ring so
the whole program fits the 8 PSUM banks alongside double-buffered score and
AV accumulators.
"""

from collections import deque

import numpy as np
import ml_dtypes

import concourse.bass as bass
import concourse.tile as tile
from concourse import bacc, library_config, mybir
from concourse.bass_utils import run_bass_kernel_spmd

B, T, C = 4, 2048, 1024
H, D = 16, 64
P = 128           # key tile size
QC = 256          # query chunk size
HC = 512          # head channels per core (8 heads x 64)
NP = 4            # head pairs per core
BF = mybir.dt.bfloat16
F32 = mybir.dt.float32
BF16NP = ml_dtypes.bfloat16
EXP = mybir.ActivationFunctionType.Exp
SCALE = float(C) ** -0.5
VS = 2 * NP * 65  # 520: per key-tile v row: 4 pairs x (2 heads x 65)


def build_kernel(nc: bass.Bass):
    xT = nc.dram_tensor("xT", [C, T], BF, kind="ExternalInput").ap()
    wq = nc.dram_tensor("wq", [C, HC], BF, kind="ExternalInput").ap()
    wk = nc.dram_tensor("wk", [C, HC], BF, kind="ExternalInput").ap()
    wv = nc.dram_tensor("wv", [C, HC], BF, kind="ExternalInput").ap()
    wp = nc.dram_tensor("wp", [HC, C], BF, kind="ExternalInput").ap()
    maskd = nc.dram_tensor("maskd", [P, 4 * QC], BF, kind="ExternalInput").ap()
    outd = nc.dram_tensor("out", [T, C], BF, kind="ExternalOutput").ap()

    with tile.TileContext(nc) as tc:
        nc.gpsimd.load_library(library_config.attn)
        with (
            tc.tile_pool(name="res", bufs=1) as rpool,
            tc.tile_pool(name="wqkv", bufs=1) as wpool,
            tc.tile_pool(name="xt", bufs=4) as xpool,
            tc.tile_pool(name="et", bufs=4) as epool,
            tc.tile_pool(name="nrm", bufs=2) as npool,
            tc.tile_pool(name="ost", bufs=2) as opool,
            tc.tile_pool(name="ps_gp", bufs=2, space="PSUM") as ps_gp,
            tc.tile_pool(name="ps_sc", bufs=2, space="PSUM") as ps_sc,
            tc.tile_pool(name="ps_av", bufs=2, space="PSUM") as ps_av,
        ):
            qT_sb = rpool.tile([P, NP * T], BF)
            kT_sb = rpool.tile([P, NP * T], BF)
            v_sb = rpool.tile([P, 16 * VS], BF)
            attn_sb = rpool.tile([P, NP * T], BF)
            wp_sb = rpool.tile([P, NP * C], BF)
            mask_sb = rpool.tile([P, 4 * QC], BF)
            wq_sb = wpool.tile([P, 8 * HC], BF, tag="wq")
            wk_sb = wpool.tile([P, 8 * HC], BF, tag="wk")
            wv_sb = wpool.tile([P, 8 * HC], BF, tag="wv")

            # DMA order = first-needed first: xt0+wq gate the first matmul.
            xts = [
                xpool.tile([P, 8 * 512], BF, tag="xt", name=f"xt{tb}")
                for tb in range(4)
            ]
            xt0_dst = xts[0][:].rearrange("p (g t) -> p g t", t=512)
            xt0_src = xT.rearrange("(g p) t -> p g t", p=P)[:, :, 0:512]
            wq_dst = wq_sb[:].rearrange("p (g h) -> p g h", h=HC)
            wq_src = wq.rearrange("(g p) h -> p g h", p=P)
            nc.sync.dma_start(xt0_dst[:, 0:4], xt0_src[:, 0:4])
            nc.sync.dma_start(wq_dst[:, :, 0:P], wq_src[:, :, 0:P])
            nc.sync.dma_start(xt0_dst[:, 4:8], xt0_src[:, 4:8])
            for hp in range(1, 4):
                nc.sync.dma_start(
                    wq_dst[:, :, hp * P:(hp + 1) * P],
                    wq_src[:, :, hp * P:(hp + 1) * P],
                )
            for tb in range(1, 4):
                nc.sync.dma_start(
                    xts[tb][:].rearrange("p (g t) -> p g t", t=512),
                    xT.rearrange("(g p) t -> p g t", p=P)
                    [:, :, tb * 512:(tb + 1) * 512],
                )
            for w_sb, w_d in ((wk_sb, wk), (wv_sb, wv)):
                nc.sync.dma_start(
                    w_sb[:].rearrange("p (g h) -> p g h", h=HC),
                    w_d.rearrange("(g p) h -> p g h", p=P),
                )
            nc.sync.dma_start(mask_sb[:], maskd[:])
            nc.sync.dma_start(
                wp_sb[:].rearrange("p (g c) -> p g c", c=C),
                wp.rearrange("(g p) c -> p g c", p=P),
            )
            nc.vector.memset(
                v_sb[:].rearrange("p (x e) -> p x e", e=65)[:, :, 64:65], 1.0
            )
            # Tiny dummy exp: pulls the ~2.7us ACT table load into the
            # prologue instead of the first attention unit.
            warm_i = npool.tile([1, 2], F32, tag="wmi")
            nc.vector.memset(warm_i[:], 0.0)
            warm_o = npool.tile([1, 2], F32, tag="wmo")
            nc.scalar.activation(warm_o[:], warm_i[:], EXP, scale=1.0)

            # ---------------- Q prologue ----------------
            for tb in range(4):
                for hp in range(NP):
                    ps = ps_gp.tile([P, 512], F32, tag="gp", name=f"q{tb}{hp}")
                    for g in range(8):
                        nc.tensor.matmul(
                            ps[:],
                            wq_sb[:, g * HC + hp * P:][:, :P],
                            xts[tb][:, g * 512:(g + 1) * 512],
                            start=(g == 0), stop=(g == 7),
                        )
                    nc.vector.tensor_copy(
                        qT_sb[:, hp * T + tb * 512:][:, :512], ps[:]
                    )

            # ---------------- filler machinery ----------------
            state = {"tb_done": -1}
            fill = []

            def kv_gen():
                for tb in range(4):
                    for hp in range(NP):
                        ps = ps_gp.tile([P, 512], F32, tag="gp",
                                        name=f"k{tb}{hp}")
                        for g in range(8):
                            nc.tensor.matmul(
                                ps[:],
                                wk_sb[:, g * HC + hp * P:][:, :P],
                                xts[tb][:, g * 512:(g + 1) * 512],
                                start=(g == 0), stop=(g == 7),
                            )
                            yield
                        nc.vector.tensor_copy(
                            kT_sb[:, hp * T + tb * 512:][:, :512], ps[:]
                        )
                        yield
                    for sj in range(4):
                        ps = ps_gp.tile([P, 512], F32, tag="gp",
                                        name=f"v{tb}{sj}")
                        for g in range(8):
                            nc.tensor.matmul(
                                ps[:],
                                xts[tb][:, g * 512 + sj * P:][:, :P],
                                wv_sb[:, g * HC:(g + 1) * HC],
                                start=(g == 0), stop=(g == 7),
                            )
                            yield
                        j = tb * 4 + sj
                        nc.vector.tensor_copy(
                            v_sb[:, j * VS:(j + 1) * VS]
                            .rearrange("p (x e) -> p x e", e=65)[:, :, 0:64],
                            ps[:].rearrange("p (x d) -> p x d", d=64),
                        )
                        yield
                    state["tb_done"] = tb

            def proj_gen(qc):
                os = opool.tile([P, 2 * C], BF, tag="os", name=f"os{qc}")
                for u2 in range(2):
                    tt = 2 * qc + u2
                    for oc in range(2):
                        pj = ps_gp.tile([P, 512], F32, tag="gp",
                                        name=f"pj{qc}{u2}{oc}")
                        for p in range(NP):
                            nc.tensor.matmul(
                                pj[:],
                                attn_sb[:, p * T + tt * P:][:, :P],
                                wp_sb[:, p * C + oc * 512:][:, :512],
                                start=(p == 0), stop=(p == 3),
                            )
                            yield
                        nc.vector.tensor_copy(
                            os[:, u2 * C + oc * 512:][:, :512], pj[:]
                        )
                        yield
                nc.sync.dma_start(
                    outd[qc * 2 * P:(qc + 1) * 2 * P, :]
                    .rearrange("(u p) c -> p u c", p=P),
                    os[:].rearrange("p (u c) -> p u c", c=C),
                )

            def pop_fill(n=1):
                for _ in range(n):
                    while fill:
                        try:
                            next(fill[0])
                            break
                        except StopIteration:
                            fill.pop(0)
                    else:
                        return

            def drain_until_tb(tb):
                while state["tb_done"] < tb:
                    try:
                        next(fill[0])
                    except StopIteration:
                        fill.pop(0)

            # ---------------- attention ----------------
            pend = deque()

            def norm(p, qc, avp):
                rs = npool.tile([1, 2 * QC], F32, tag="rs")
                nc.vector.tensor_copy(rs[:], avp[64:65, :])
                avc = npool.tile([64, 2 * QC], F32, tag="avc")
                nc.vector.tensor_copy(avc[:], avp[0:64, :])
                rc = npool.tile([1, 2 * QC], F32, tag="rc")
                nc.vector.reciprocal_approx_fast(rc[:], rs[:])
                rb = npool.tile([64, 2 * QC], F32, tag="rb")
                nc.gpsimd.partition_broadcast(rb[:], rc[:])
                col = p * T + qc * QC
                nc.vector.tensor_mul(
                    attn_sb[0:64, col:col + QC], avc[:, 0:QC], rb[:, 0:QC]
                )
                nc.vector.tensor_mul(
                    attn_sb[64:128, col:col + QC],
                    avc[:, QC:2 * QC], rb[:, QC:2 * QC],
                )

            def emit_av(pv):
                e_t, avp, p, j0, nk, cb = pv
                j1 = j0 + 1
                base0 = j0 * VS + p * 130
                base1 = j1 * VS + p * 130
                last = (j1 == nk - 1)
                # avp is a single PSUM bank: exactly one start (first mm of
                # the bank) and one stop (last mm of the bank).
                nc.tensor.matmul(avp[:, 0:QC], v_sb[:, base0:base0 + 65],
                                 e_t[:, 0:QC],
                                 start=(j0 == 0), stop=False)
                nc.tensor.matmul(avp[:, 0:QC], v_sb[:, base1:base1 + 65],
                                 e_t[:, QC:2 * QC],
                                 start=False, stop=False)
                nc.tensor.matmul(avp[:, QC:2 * QC],
                                 v_sb[:, base0 + 65:base0 + 130],
                                 e_t[:, 2 * QC:3 * QC],
                                 start=False, stop=False)
                nc.tensor.matmul(avp[:, QC:2 * QC],
                                 v_sb[:, base1 + 65:base1 + 130],
                                 e_t[:, 3 * QC:4 * QC],
                                 start=False, stop=last)
                if last and cb is not None:
                    cb()

            def c_run(p, qc):
                nk = 2 * (qc + 1)
                avp = ps_av.tile([65, 2 * QC], F32, tag="av",
                                 name=f"av{p}_{qc}")
                qA = qT_sb[0:64, p * T + qc * QC:][:, :QC]
                qB = qT_sb[64:128, p * T + qc * QC:][:, :QC]

                def cb(pp=p, qq=qc, aa=avp):
                    norm(pp, qq, aa)
                    if pp == 3:
                        fill.append(proj_gen(qq))

                for u in range(qc + 1):
                    j0, j1 = 2 * u, 2 * u + 1
                    kt0 = kT_sb[:, p * T + j0 * P:][:, :P]
                    kt1 = kT_sb[:, p * T + j1 * P:][:, :P]
                    # sc spans 2 PSUM banks: head A cols [0:2QC] (bank 0),
                    # head B cols [2QC:4QC] (bank 1) -> concurrent row-group
                    # matmuls land in different banks; one start/stop per bank.
                    sc = ps_sc.tile([P, 4 * QC], F32, tag="sc")
                    nc.tensor.matmul(sc[:, 0:QC], kt0[0:64, :], qA,
                                     start=True, stop=False,
                                     tile_position=(0, 0))
                    nc.tensor.matmul(sc[:, 2 * QC:3 * QC], kt0[64:128, :],
                                     qB, start=True, stop=False,
                                     tile_position=(64, 0))
                    nc.tensor.matmul(sc[:, QC:2 * QC], kt1[0:64, :], qA,
                                     start=False, stop=True,
                                     tile_position=(0, 0))
                    nc.tensor.matmul(sc[:, 3 * QC:4 * QC], kt1[64:128, :],
                                     qB, start=False, stop=True,
                                     tile_position=(64, 0))
                    e_t = epool.tile([P, 4 * QC], BF, tag="exp")
                    nc.scalar.activation(e_t[:], sc[:], EXP, scale=SCALE)
                    if u == qc:
                        e_m = epool.tile([P, 4 * QC], BF, tag="expm", bufs=2)
                        nc.vector.tensor_mul(e_m[:], e_t[:], mask_sb[:])
                        e_t = e_m
                    # Fillers BEFORE the AV emission: if the pending unit's
                    # exp/mask isn't done yet, the filler matmuls keep the PE
                    # queue moving instead of stalling behind the AV wait.
                    pop_fill(2)
                    if len(pend) >= 2:
                        emit_av(pend.popleft())
                    pend.append((e_t, avp, p, j0, nk,
                                 cb if u == qc else None))

            fill.append(kv_gen())
            for qc in range(8):
                drain_until_tb(qc // 2)
                for p in range(NP):
                    c_run(p, qc)
            while pend:
                pop_fill(2)
                emit_av(pend.popleft())
            pop_fill(10 ** 6)
    return nc


def _make_mask():
    s = np.arange(P)[:, None]
    t = np.arange(QC)[None, :]
    m0 = (s <= t).astype(np.float32)
    m1 = (s <= t - P).astype(np.float32)
    return np.ascontiguousarray(
        np.concatenate([m0, m1, m0, m1], axis=1).astype(BF16NP)
    )


_CACHE = {}


def _get_nc():
    if "nc" not in _CACHE:
        nc = bacc.Bacc("TRN2", target_bir_lowering=False, debug=False)
        build_kernel(nc)
        nc.compile()
        _CACHE["nc"] = nc
    return _CACHE["nc"]


def make_in_maps(x, wq, wk, wv, w_proj, b_proj):
    x = np.asarray(x, np.float32)
    wq = np.asarray(wq, np.float32)
    wk = np.asarray(wk, np.float32)
    wv = np.asarray(wv, np.float32)
    w_proj = np.asarray(w_proj, np.float32)
    mask = _make_mask()

    halves = []
    for hh in range(2):
        hs = slice(hh * 8, hh * 8 + 8)
        halves.append({
            "wq": np.ascontiguousarray(
                np.transpose(wq[hs], (1, 0, 2)).reshape(C, HC).astype(BF16NP)),
            "wk": np.ascontiguousarray(
                np.transpose(wk[hs], (1, 0, 2)).reshape(C, HC).astype(BF16NP)),
            "wv": np.ascontiguousarray(
                np.transpose(wv[hs], (1, 0, 2)).reshape(C, HC).astype(BF16NP)),
            "wp": np.ascontiguousarray(
                w_proj[hh * HC:(hh + 1) * HC, :].astype(BF16NP)),
        })
    xTs = [np.ascontiguousarray(x[b].T.astype(BF16NP)) for b in range(B)]

    in_maps = []
    for core in range(8):
        b, hh = core // 2, core % 2
        w = halves[hh]
        in_maps.append({
            "xT": xTs[b],
            "wq": w["wq"], "wk": w["wk"], "wv": w["wv"], "wp": w["wp"],
            "maskd": mask,
        })
    return in_maps


def assemble(results, b_proj):
    bias = np.asarray(b_proj, np.float32)
    full = np.empty((B, T, C), np.float32)
    for b in range(B):
        p0 = np.asarray(results[2 * b]["out"]).astype(np.float32)
        p1 = np.asarray(results[2 * b + 1]["out"]).astype(np.float32)
        full[b] = p0 + p1 + bias[None, :]
    return full


def kernel(x, wq, wk, wv, w_proj, b_proj, _trace=False, _tmpdir=None):
    in_maps = make_in_maps(x, wq, wk, wv, w_proj, b_proj)
    nc = _get_nc()
    res = run_bass_kernel_spmd(
        nc, in_maps, core_ids=list(range(8)), trace=_trace, tmpdir=_tmpdir
    )
    if _trace:
        _CACHE["last_result"] = res
    return assemble(res.results, b_proj)


# revision 12
# speedup vs baseline: 2.2454x; 1.0016x over previous
"""Multi-head causal attention (B=4,T=2048,C=1024,H=16,D=64) on 8 TRN2 NeuronCores.

Sharding: batch x head-half tensor parallel. Core c handles batch b=c//2 and
heads [8*(c%2), 8*(c%2)+8) over ALL 2048 queries. Each core computes its own
Q/K/V projections (no duplicated work, no cross-core traffic), causal
attention for its 8 heads, and a partial output projection
attn_half @ w_proj[half_rows]. The host unshards by summing the two partials
per batch and adding the bias (the all-reduce of the TP sharding, done at
gather time). Causal load is uniform per core by construction.

Per-core program (bf16 matmuls, fp32 PSUM). The scalar engine's exp stream is
the scarce resource (~190us), so the program is a single software-pipelined
stream that keeps it fed from ~15% in:
  1. Q projections for all 2048 queries up front (PE-dense prologue).
  2. K/V projections are chopped into per-matmul "filler" generators,
     interleaved a couple of matmuls per attention unit so the PE stays busy
     while the scalar engine paces the exp stream; a query chunk's attention
     starts as soon as its causal key prefix is projected.
  3. Attention unit (head-pair, 256-query chunk, PAIR of 128-key tiles):
     scores into a 2-bank PSUM tile (2 heads via PE row groups -> separate
     banks; one start/stop per bank) -> exp on ACT (scale folded) -> 0/1
     causal mask on DVE (diagonal unit only) -> wei @ [v|1] accumulated in
     PSUM ([65,256]/head, row 64 = sumexp). AV emission is delayed two units
     so the PE never waits on ACT. Normalization: DVE fast reciprocal +
     GPSIMD partition broadcast.
  4. The partial projection for a query chunk is queued as another filler
     generator once its last head-pair is normalized; per-chunk output DMA.
All dense-GEMM PSUM tiles (Q/K/V/proj) share one 2-buffer [128,512] ring so
the whole program fits the 8 PSUM banks alongside double-buffered score and
AV accumulators.
"""

from collections import deque

import numpy as np
import ml_dtypes

import concourse.bass as bass
import concourse.tile as tile
from concourse import bacc, library_config, mybir
from concourse.bass_utils import run_bass_kernel_spmd

B, T, C = 4, 2048, 1024
H, D = 16, 64
P = 128           # key tile size
QC = 256          # query chunk size
HC = 512          # head channels per core (8 heads x 64)
NP = 4            # head pairs per core
BF = mybir.dt.bfloat16
F32 = mybir.dt.float32
BF16NP = ml_dtypes.bfloat16
EXP = mybir.ActivationFunctionType.Exp
SCALE = float(C) ** -0.5
VS = 2 * NP * 65  # 520: per key-tile v row: 4 pairs x (2 heads x 65)


def build_kernel(nc: bass.Bass):
    xT = nc.dram_tensor("xT", [C, T], BF, kind="ExternalInput").ap()
    wq = nc.dram_tensor("wq", [C, HC], BF, kind="ExternalInput").ap()
    wk = nc.dram_tensor("wk", [C, HC], BF, kind="ExternalInput").ap()
    wv = nc.dram_tensor("wv", [C, HC], BF, kind="ExternalInput").ap()
    wp = nc.dram_tensor("wp", [HC, C], BF, kind="ExternalInput").ap()
    maskd = nc.dram_tensor("maskd", [P, 4 * QC], BF, kind="ExternalInput").ap()
    outd = nc.dram_tensor("out", [T, C], BF, kind="ExternalOutput").ap()

    with tile.TileContext(nc) as tc:
        nc.gpsimd.load_library(library_config.attn)
        with (
            tc.tile_pool(name="res", bufs=1) as rpool,
            tc.tile_pool(name="wqkv", bufs=1) as wpool,
            tc.tile_pool(name="xt", bufs=4) as xpool,
            tc.tile_pool(name="et", bufs=4) as epool,
            tc.tile_pool(name="nrm", bufs=2) as npool,
            tc.tile_pool(name="ost", bufs=2) as opool,
            tc.tile_pool(name="ps_gp", bufs=2, space="PSUM") as ps_gp,
            tc.tile_pool(name="ps_sc", bufs=2, space="PSUM") as ps_sc,
            tc.tile_pool(name="ps_av", bufs=2, space="PSUM") as ps_av,
        ):
            # PE warmup: ~60 dummy matmuls on memset data fill the ~13us
            # input-DMA window so the HAM clock gate reaches 8/8 before the
            # first real matmul (and never sees a >3.4us idle window).
            warm_w = rpool.tile([P, 256], BF)
            nc.vector.memset(warm_w[:], 0.0)
            wps = ps_gp.tile([P, 512], F32, tag="gp", name="warmps")
            for i in range(60):
                nc.tensor.matmul(wps[:, 0:256], warm_w[:, 0:128], warm_w[:],
                                 start=(i == 0), stop=(i == 59))

            qT_sb = rpool.tile([P, NP * T], BF)
            kT_sb = rpool.tile([P, NP * T], BF)
            v_sb = rpool.tile([P, 16 * VS], BF)
            attn_sb = rpool.tile([P, NP * T], BF)
            wp_sb = rpool.tile([P, NP * C], BF)
            mask_sb = rpool.tile([P, 4 * QC], BF)
            wq_sb = wpool.tile([P, 8 * HC], BF, tag="wq")
            wk_sb = wpool.tile([P, 8 * HC], BF, tag="wk")
            wv_sb = wpool.tile([P, 8 * HC], BF, tag="wv")

            # DMA order = first-needed first: xt0+wq gate the first matmul.
            xts = [
                xpool.tile([P, 8 * 512], BF, tag="xt", name=f"xt{tb}")
                for tb in range(4)
            ]
            xt0_dst = xts[0][:].rearrange("p (g t) -> p g t", t=512)
            xt0_src = xT.rearrange("(g p) t -> p g t", p=P)[:, :, 0:512]
            wq_dst = wq_sb[:].rearrange("p (g h) -> p g h", h=HC)
            wq_src = wq.rearrange("(g p) h -> p g h", p=P)
            nc.sync.dma_start(xt0_dst[:, 0:4], xt0_src[:, 0:4])
            nc.sync.dma_start(wq_dst[:, :, 0:P], wq_src[:, :, 0:P])
            nc.sync.dma_start(xt0_dst[:, 4:8], xt0_src[:, 4:8])
            for hp in range(1, 4):
                nc.sync.dma_start(
                    wq_dst[:, :, hp * P:(hp + 1) * P],
                    wq_src[:, :, hp * P:(hp + 1) * P],
                )
            for tb in range(1, 4):
                nc.sync.dma_start(
                    xts[tb][:].rearrange("p (g t) -> p g t", t=512),
                    xT.rearrange("(g p) t -> p g t", p=P)
                    [:, :, tb * 512:(tb + 1) * 512],
                )
            for w_sb, w_d in ((wk_sb, wk), (wv_sb, wv)):
                nc.sync.dma_start(
                    w_sb[:].rearrange("p (g h) -> p g h", h=HC),
                    w_d.rearrange("(g p) h -> p g h", p=P),
                )
            nc.sync.dma_start(mask_sb[:], maskd[:])
            nc.sync.dma_start(
                wp_sb[:].rearrange("p (g c) -> p g c", c=C),
                wp.rearrange("(g p) c -> p g c", p=P),
            )
            nc.vector.memset(
                v_sb[:].rearrange("p (x e) -> p x e", e=65)[:, :, 64:65], 1.0
            )
            # Tiny dummy exp: pulls the ~2.7us ACT table load into the
            # prologue instead of the first attention unit.
            warm_i = npool.tile([1, 2], F32, tag="wmi")
            nc.vector.memset(warm_i[:], 0.0)
            warm_o = npool.tile([1, 2], F32, tag="wmo")
            nc.scalar.activation(warm_o[:], warm_i[:], EXP, scale=1.0)

            # ---------------- Q prologue ----------------
            for tb in range(4):
                for hp in range(NP):
                    ps = ps_gp.tile([P, 512], F32, tag="gp", name=f"q{tb}{hp}")
                    for g in range(8):
                        nc.tensor.matmul(
                            ps[:],
                            wq_sb[:, g * HC + hp * P:][:, :P],
                            xts[tb][:, g * 512:(g + 1) * 512],
                            start=(g == 0), stop=(g == 7),
                        )
                    nc.vector.tensor_copy(
                        qT_sb[:, hp * T + tb * 512:][:, :512], ps[:]
                    )

            # ---------------- filler machinery ----------------
            state = {"tb_done": -1}
            fill = []

            def kv_gen():
                for tb in range(4):
                    for hp in range(NP):
                        ps = ps_gp.tile([P, 512], F32, tag="gp",
                                        name=f"k{tb}{hp}")
                        for g in range(8):
                            nc.tensor.matmul(
                                ps[:],
                                wk_sb[:, g * HC + hp * P:][:, :P],
                                xts[tb][:, g * 512:(g + 1) * 512],
                                start=(g == 0), stop=(g == 7),
                            )
                            yield
                        nc.vector.tensor_copy(
                            kT_sb[:, hp * T + tb * 512:][:, :512], ps[:]
                        )
                        yield
                    for sj in range(4):
                        ps = ps_gp.tile([P, 512], F32, tag="gp",
                                        name=f"v{tb}{sj}")
                        for g in range(8):
                            nc.tensor.matmul(
                                ps[:],
                                xts[tb][:, g * 512 + sj * P:][:, :P],
                                wv_sb[:, g * HC:(g + 1) * HC],
                                start=(g == 0), stop=(g == 7),
                            )
                            yield
                        j = tb * 4 + sj
                        nc.vector.tensor_copy(
                            v_sb[:, j * VS:(j + 1) * VS]
                            .rearrange("p (x e) -> p x e", e=65)[:, :, 0:64],
                            ps[:].rearrange("p (x d) -> p x d", d=64),
                        )
                        yield
                    state["tb_done"] = tb

            def proj_gen(qc):
                os = opool.tile([P, 2 * C], BF, tag="os", name=f"os{qc}")
                for u2 in range(2):
                    tt = 2 * qc + u2
                    for oc in range(2):
                        pj = ps_gp.tile([P, 512], F32, tag="gp",
                                        name=f"pj{qc}{u2}{oc}")
                        for p in range(NP):
                            nc.tensor.matmul(
                                pj[:],
                                attn_sb[:, p * T + tt * P:][:, :P],
                                wp_sb[:, p * C + oc * 512:][:, :512],
                                start=(p == 0), stop=(p == 3),
                            )
                            yield
                        nc.vector.tensor_copy(
                            os[:, u2 * C + oc * 512:][:, :512], pj[:]
                        )
                        yield
                nc.sync.dma_start(
                    outd[qc * 2 * P:(qc + 1) * 2 * P, :]
                    .rearrange("(u p) c -> p u c", p=P),
                    os[:].rearrange("p (u c) -> p u c", c=C),
                )

            def pop_fill(n=1):
                for _ in range(n):
                    while fill:
                        try:
                            next(fill[0])
                            break
                        except StopIteration:
                            fill.pop(0)
                    else:
                        return

            def drain_until_tb(tb):
                while state["tb_done"] < tb:
                    try:
                        next(fill[0])
                    except StopIteration:
                        fill.pop(0)

            # ---------------- attention ----------------
            pend = deque()

            def norm(p, qc, avp):
                rs = npool.tile([1, 2 * QC], F32, tag="rs")
                nc.vector.tensor_copy(rs[:], avp[64:65, :])
                avc = npool.tile([64, 2 * QC], F32, tag="avc")
                nc.vector.tensor_copy(avc[:], avp[0:64, :])
                rc = npool.tile([1, 2 * QC], F32, tag="rc")
                nc.vector.reciprocal_approx_fast(rc[:], rs[:])
                rb = npool.tile([64, 2 * QC], F32, tag="rb")
                nc.gpsimd.partition_broadcast(rb[:], rc[:])
                col = p * T + qc * QC
                nc.vector.tensor_mul(
                    attn_sb[0:64, col:col + QC], avc[:, 0:QC], rb[:, 0:QC]
                )
                nc.vector.tensor_mul(
                    attn_sb[64:128, col:col + QC],
                    avc[:, QC:2 * QC], rb[:, QC:2 * QC],
                )

            def emit_av(pv):
                e_t, avp, p, j0, nk, cb = pv
                j1 = j0 + 1
                base0 = j0 * VS + p * 130
                base1 = j1 * VS + p * 130
                last = (j1 == nk - 1)
                # avp is a single PSUM bank: exactly one start (first mm of
                # the bank) and one stop (last mm of the bank).
                nc.tensor.matmul(avp[:, 0:QC], v_sb[:, base0:base0 + 65],
                                 e_t[:, 0:QC],
                                 start=(j0 == 0), stop=False)
                nc.tensor.matmul(avp[:, 0:QC], v_sb[:, base1:base1 + 65],
                                 e_t[:, QC:2 * QC],
                                 start=False, stop=False)
                nc.tensor.matmul(avp[:, QC:2 * QC],
                                 v_sb[:, base0 + 65:base0 + 130],
                                 e_t[:, 2 * QC:3 * QC],
                                 start=False, stop=False)
                nc.tensor.matmul(avp[:, QC:2 * QC],
                                 v_sb[:, base1 + 65:base1 + 130],
                                 e_t[:, 3 * QC:4 * QC],
                                 start=False, stop=last)
                if last and cb is not None:
                    cb()

            def c_run(p, qc):
                nk = 2 * (qc + 1)
                avp = ps_av.tile([65, 2 * QC], F32, tag="av",
                                 name=f"av{p}_{qc}")
                qA = qT_sb[0:64, p * T + qc * QC:][:, :QC]
                qB = qT_sb[64:128, p * T + qc * QC:][:, :QC]

                def cb(pp=p, qq=qc, aa=avp):
                    norm(pp, qq, aa)
                    if pp == 3:
                        fill.append(proj_gen(qq))

                for u in range(qc + 1):
                    j0, j1 = 2 * u, 2 * u + 1
                    kt0 = kT_sb[:, p * T + j0 * P:][:, :P]
                    kt1 = kT_sb[:, p * T + j1 * P:][:, :P]
                    # sc spans 2 PSUM banks: head A cols [0:2QC] (bank 0),
                    # head B cols [2QC:4QC] (bank 1) -> concurrent row-group
                    # matmuls land in different banks; one start/stop per bank.
                    sc = ps_sc.tile([P, 4 * QC], F32, tag="sc")
                    nc.tensor.matmul(sc[:, 0:QC], kt0[0:64, :], qA,
                                     start=True, stop=False,
                                     tile_position=(0, 0))
                    nc.tensor.matmul(sc[:, 2 * QC:3 * QC], kt0[64:128, :],
                                     qB, start=True, stop=False,
                                     tile_position=(64, 0))
                    nc.tensor.matmul(sc[:, QC:2 * QC], kt1[0:64, :], qA,
                                     start=False, stop=True,
                                     tile_position=(0, 0))
                    nc.tensor.matmul(sc[:, 3 * QC:4 * QC], kt1[64:128, :],
                                     qB, start=False, stop=True,
                                     tile_position=(64, 0))
                    e_t = epool.tile([P, 4 * QC], BF, tag="exp")
                    nc.scalar.activation(e_t[:], sc[:], EXP, scale=SCALE)
                    if u == qc:
                        e_m = epool.tile([P, 4 * QC], BF, tag="expm", bufs=2)
                        nc.vector.tensor_mul(e_m[:], e_t[:], mask_sb[:])
                        e_t = e_m
                    # Fillers BEFORE the AV emission: if the pending unit's
                    # exp/mask isn't done yet, the filler matmuls keep the PE
                    # queue moving instead of stalling behind the AV wait.
                    pop_fill(2)
                    if len(pend) >= 2:
                        emit_av(pend.popleft())
                    pend.append((e_t, avp, p, j0, nk,
                                 cb if u == qc else None))

            fill.append(kv_gen())
            for qc in range(8):
                drain_until_tb(qc // 2)
                for p in range(NP):
                    c_run(p, qc)
            while pend:
                pop_fill(2)
                emit_av(pend.popleft())
            pop_fill(10 ** 6)
    return nc


def _make_mask():
    s = np.arange(P)[:, None]
    t = np.arange(QC)[None, :]
    m0 = (s <= t).astype(np.float32)
    m1 = (s <= t - P).astype(np.float32)
    return np.ascontiguousarray(
        np.concatenate([m0, m1, m0, m1], axis=1).astype(BF16NP)
    )


_CACHE = {}


def _get_nc():
    if "nc" not in _CACHE:
        nc = bacc.Bacc("TRN2", target_bir_lowering=False, debug=False)
        build_kernel(nc)
        nc.compile()
        _CACHE["nc"] = nc
    return _CACHE["nc"]


def make_in_maps(x, wq, wk, wv, w_proj, b_proj):
    x = np.asarray(x, np.float32)
    wq = np.asarray(wq, np.float32)
    wk = np.asarray(wk, np.float32)
    wv = np.asarray(wv, np.float32)
    w_proj = np.asarray(w_proj, np.float32)
    mask = _make_mask()

    halves = []
    for hh in range(2):
        hs = slice(hh * 8, hh * 8 + 8)
        halves.append({
            "wq": np.ascontiguousarray(
                np.transpose(wq[hs], (1, 0, 2)).reshape(C, HC).astype(BF16NP)),
            "wk": np.ascontiguousarray(
                np.transpose(wk[hs], (1, 0, 2)).reshape(C, HC).astype(BF16NP)),
            "wv": np.ascontiguousarray(
                np.transpose(wv[hs], (1, 0, 2)).reshape(C, HC).astype(BF16NP)),
            "wp": np.ascontiguousarray(
                w_proj[hh * HC:(hh + 1) * HC, :].astype(BF16NP)),
        })
    xTs = [np.ascontiguousarray(x[b].T.astype(BF16NP)) for b in range(B)]

    in_maps = []
    for core in range(8):
        b, hh = core // 2, core % 2
        w = halves[hh]
        in_maps.append({
            "xT": xTs[b],
            "wq": w["wq"], "wk": w["wk"], "wv": w["wv"], "wp": w["wp"],
            "maskd": mask,
        })
    return in_maps


def assemble(results, b_proj):
    bias = np.asarray(b_proj, np.float32)
    full = np.empty((B, T, C), np.float32)
    for b in range(B):
        p0 = np.asarray(results[2 * b]["out"]).astype(np.float32)
        p1 = np.asarray(results[2 * b + 1]["out"]).astype(np.float32)
        full[b] = p0 + p1 + bias[None, :]
    return full


def kernel(x, wq, wk, wv, w_proj, b_proj, _trace=False, _tmpdir=None):
    in_maps = make_in_maps(x, wq, wk, wv, w_proj, b_proj)
    nc = _get_nc()
    res = run_bass_kernel_spmd(
        nc, in_maps, core_ids=list(range(8)), trace=_trace, tmpdir=_tmpdir
    )
    if _trace:
        _CACHE["last_result"] = res
    return assemble(res.results, b_proj)


# revision 14
# speedup vs baseline: 2.3292x; 1.0373x over previous
"""Multi-head causal attention (B=4,T=2048,C=1024,H=16,D=64) on 8 TRN2 NeuronCores.

Sharding: batch x head-half tensor parallel. Core c handles batch b=c//2 and
heads [8*(c%2), 8*(c%2)+8) over ALL 2048 queries. Each core computes its own
Q/K/V projections (no duplicated work, no cross-core traffic), causal
attention for its 8 heads, and a partial output projection
attn_half @ w_proj[half_rows]. The host unshards by summing the two partials
per batch and adding the bias (the all-reduce of the TP sharding, done at
gather time). Causal load is uniform per core by construction.

Per-core program (bf16 matmuls, fp32 PSUM). The scalar engine's exp stream is
the scarce resource (~190us), so the program is a single software-pipelined
stream that keeps it fed from ~15% in:
  1. Q projections for all 2048 queries up front (PE-dense prologue).
  2. K/V projections are chopped into per-matmul "filler" generators,
     interleaved a couple of matmuls per attention unit so the PE stays busy
     while the scalar engine paces the exp stream; a query chunk's attention
     starts as soon as its causal key prefix is projected.
  3. Attention unit (head-pair, 256-query chunk, PAIR of 128-key tiles):
     scores into a 2-bank PSUM tile (2 heads via PE row groups -> separate
     banks; one start/stop per bank) -> exp on ACT (scale folded) -> 0/1
     causal mask on DVE (diagonal unit only) -> wei @ [v|1] accumulated in
     PSUM ([65,256]/head, row 64 = sumexp). AV emission is delayed two units
     so the PE never waits on ACT. Normalization: DVE fast reciprocal +
     GPSIMD partition broadcast.
  4. The partial projection for a query chunk is queued as another filler
     generator once its last head-pair is normalized; per-chunk output DMA.
All dense-GEMM PSUM tiles (Q/K/V/proj) share one 2-buffer [128,512] ring so
the whole program fits the 8 PSUM banks alongside double-buffered score and
AV accumulators.
"""

from collections import deque

import numpy as np
import ml_dtypes

import concourse.bass as bass
import concourse.tile as tile
from concourse import bacc, library_config, mybir
from concourse.bass_utils import run_bass_kernel_spmd

B, T, C = 4, 2048, 1024
H, D = 16, 64
P = 128           # key tile size
QC = 256          # query chunk size
HC = 512          # head channels per core (8 heads x 64)
NP = 4            # head pairs per core
BF = mybir.dt.bfloat16
F32 = mybir.dt.float32
BF16NP = ml_dtypes.bfloat16
EXP = mybir.ActivationFunctionType.Exp
SCALE = float(C) ** -0.5
VS = 2 * NP * 65  # 520: per key-tile v row: 4 pairs x (2 heads x 65)


def build_kernel(nc: bass.Bass):
    xT = nc.dram_tensor("xT", [C, T], BF, kind="ExternalInput").ap()
    wq = nc.dram_tensor("wq", [C, HC], BF, kind="ExternalInput").ap()
    wk = nc.dram_tensor("wk", [C, HC], BF, kind="ExternalInput").ap()
    wv = nc.dram_tensor("wv", [C, HC], BF, kind="ExternalInput").ap()
    wp = nc.dram_tensor("wp", [HC, C], BF, kind="ExternalInput").ap()
    maskd = nc.dram_tensor("maskd", [P, 4 * QC], BF, kind="ExternalInput").ap()
    outd = nc.dram_tensor("out", [T, C], BF, kind="ExternalOutput").ap()

    with tile.TileContext(nc) as tc:
        nc.gpsimd.load_library(library_config.attn)
        with (
            tc.tile_pool(name="res", bufs=1) as rpool,
            tc.tile_pool(name="wqkv", bufs=1) as wpool,
            tc.tile_pool(name="xt", bufs=4) as xpool,
            tc.tile_pool(name="et", bufs=4) as epool,
            tc.tile_pool(name="nrm", bufs=2) as npool,
            tc.tile_pool(name="ost", bufs=2) as opool,
            tc.tile_pool(name="ps_gp", bufs=2, space="PSUM") as ps_gp,
            tc.tile_pool(name="ps_sc", bufs=2, space="PSUM") as ps_sc,
            tc.tile_pool(name="ps_av", bufs=2, space="PSUM") as ps_av,
        ):
            # PE warmup: ~60 dummy matmuls on memset data fill the ~13us
            # input-DMA window so the HAM clock gate reaches 8/8 before the
            # first real matmul (and never sees a >3.4us idle window).
            warm_w = rpool.tile([P, 256], BF)
            nc.vector.memset(warm_w[:], 0.0)
            wps = ps_gp.tile([P, 512], F32, tag="gp", name="warmps")
            for i in range(60):
                nc.tensor.matmul(wps[:, 0:256], warm_w[:, 0:128], warm_w[:],
                                 start=(i == 0), stop=(i == 59))

            qT_sb = rpool.tile([P, NP * T], BF)
            kT_sb = rpool.tile([P, NP * T], BF)
            v_sb = rpool.tile([P, 16 * VS], BF)
            attn_sb = rpool.tile([P, NP * T], BF)
            wp_sb = rpool.tile([P, NP * C], BF)
            mask_sb = rpool.tile([P, 4 * QC], BF)
            wq_sb = wpool.tile([P, 8 * HC], BF, tag="wq")
            wk_sb = wpool.tile([P, 8 * HC], BF, tag="wk")
            wv_sb = wpool.tile([P, 8 * HC], BF, tag="wv")

            # DMA order = first-needed first: xt0+wq gate the first matmul.
            xts = [
                xpool.tile([P, 8 * 512], BF, tag="xt", name=f"xt{tb}")
                for tb in range(4)
            ]
            xt0_dst = xts[0][:].rearrange("p (g t) -> p g t", t=512)
            xt0_src = xT.rearrange("(g p) t -> p g t", p=P)[:, :, 0:512]
            wq_dst = wq_sb[:].rearrange("p (g h) -> p g h", h=HC)
            wq_src = wq.rearrange("(g p) h -> p g h", p=P)
            nc.sync.dma_start(xt0_dst[:, 0:4], xt0_src[:, 0:4])
            nc.sync.dma_start(wq_dst[:, :, 0:P], wq_src[:, :, 0:P])
            nc.sync.dma_start(xt0_dst[:, 4:8], xt0_src[:, 4:8])
            for hp in range(1, 4):
                nc.sync.dma_start(
                    wq_dst[:, :, hp * P:(hp + 1) * P],
                    wq_src[:, :, hp * P:(hp + 1) * P],
                )
            for tb in range(1, 4):
                nc.sync.dma_start(
                    xts[tb][:].rearrange("p (g t) -> p g t", t=512),
                    xT.rearrange("(g p) t -> p g t", p=P)
                    [:, :, tb * 512:(tb + 1) * 512],
                )
            for w_sb, w_d in ((wk_sb, wk), (wv_sb, wv)):
                nc.sync.dma_start(
                    w_sb[:].rearrange("p (g h) -> p g h", h=HC),
                    w_d.rearrange("(g p) h -> p g h", p=P),
                )
            nc.sync.dma_start(mask_sb[:], maskd[:])
            nc.sync.dma_start(
                wp_sb[:].rearrange("p (g c) -> p g c", c=C),
                wp.rearrange("(g p) c -> p g c", p=P),
            )
            nc.vector.memset(
                v_sb[:].rearrange("p (x e) -> p x e", e=65)[:, :, 64:65], 1.0
            )
            # Tiny dummy exp: pulls the ~2.7us ACT table load into the
            # prologue instead of the first attention unit.
            warm_i = npool.tile([1, 2], F32, tag="wmi")
            nc.vector.memset(warm_i[:], 0.0)
            warm_o = npool.tile([1, 2], F32, tag="wmo")
            nc.scalar.activation(warm_o[:], warm_i[:], EXP, scale=1.0)

            # ---------------- Q prologue ----------------
            for tb in range(4):
                for hp in range(NP):
                    ps = ps_gp.tile([P, 512], F32, tag="gp", name=f"q{tb}{hp}")
                    for g in range(8):
                        nc.tensor.matmul(
                            ps[:],
                            wq_sb[:, g * HC + hp * P:][:, :P],
                            xts[tb][:, g * 512:(g + 1) * 512],
                            start=(g == 0), stop=(g == 7),
                        )
                    nc.vector.tensor_copy(
                        qT_sb[:, hp * T + tb * 512:][:, :512], ps[:]
                    )

            # ---------------- filler machinery ----------------
            state = {"tb_done": -1}
            fill = []

            def kv_gen():
                for tb in range(4):
                    for hp in range(NP):
                        ps = ps_gp.tile([P, 512], F32, tag="gp",
                                        name=f"k{tb}{hp}")
                        for g in range(8):
                            nc.tensor.matmul(
                                ps[:],
                                wk_sb[:, g * HC + hp * P:][:, :P],
                                xts[tb][:, g * 512:(g + 1) * 512],
                                start=(g == 0), stop=(g == 7),
                            )
                            yield
                        nc.vector.tensor_copy(
                            kT_sb[:, hp * T + tb * 512:][:, :512], ps[:]
                        )
                        yield
                    for sj in range(4):
                        ps = ps_gp.tile([P, 512], F32, tag="gp",
                                        name=f"v{tb}{sj}")
                        for g in range(8):
                            nc.tensor.matmul(
                                ps[:],
                                xts[tb][:, g * 512 + sj * P:][:, :P],
                                wv_sb[:, g * HC:(g + 1) * HC],
                                start=(g == 0), stop=(g == 7),
                            )
                            yield
                        j = tb * 4 + sj
                        nc.vector.tensor_copy(
                            v_sb[:, j * VS:(j + 1) * VS]
                            .rearrange("p (x e) -> p x e", e=65)[:, :, 0:64],
                            ps[:].rearrange("p (x d) -> p x d", d=64),
                        )
                        yield
                    state["tb_done"] = tb

            def proj_gen(qc):
                os = opool.tile([P, 2 * C], BF, tag="os", name=f"os{qc}")
                for u2 in range(2):
                    tt = 2 * qc + u2
                    for oc in range(2):
                        pj = ps_gp.tile([P, 512], F32, tag="gp",
                                        name=f"pj{qc}{u2}{oc}")
                        for p in range(NP):
                            nc.tensor.matmul(
                                pj[:],
                                attn_sb[:, p * T + tt * P:][:, :P],
                                wp_sb[:, p * C + oc * 512:][:, :512],
                                start=(p == 0), stop=(p == 3),
                            )
                            yield
                        nc.vector.tensor_copy(
                            os[:, u2 * C + oc * 512:][:, :512], pj[:]
                        )
                        yield
                    nc.sync.dma_start(
                        outd[tt * P:(tt + 1) * P, :],
                        os[:, u2 * C:(u2 + 1) * C],
                    )

            def pop_fill(n=1):
                for _ in range(n):
                    while fill:
                        try:
                            next(fill[0])
                            break
                        except StopIteration:
                            fill.pop(0)
                    else:
                        return

            def drain_until_tb(tb):
                while state["tb_done"] < tb:
                    try:
                        next(fill[0])
                    except StopIteration:
                        fill.pop(0)

            # ---------------- attention ----------------
            pend = deque()

            def norm(p, qc, avp):
                rs = npool.tile([1, 2 * QC], F32, tag="rs")
                nc.vector.tensor_copy(rs[:], avp[64:65, :])
                rc = npool.tile([1, 2 * QC], F32, tag="rc")
                nc.vector.reciprocal_approx_fast(rc[:], rs[:])
                rb = npool.tile([64, 2 * QC], F32, tag="rb")
                nc.gpsimd.partition_broadcast(rb[:], rc[:])
                col = p * T + qc * QC
                nc.vector.tensor_mul(
                    attn_sb[0:64, col:col + QC], avp[0:64, 0:QC], rb[:, 0:QC]
                )
                nc.vector.tensor_mul(
                    attn_sb[64:128, col:col + QC],
                    avp[0:64, QC:2 * QC], rb[:, QC:2 * QC],
                )

            def emit_av(pv):
                e_t, avp, p, j0, nk, cb = pv
                j1 = j0 + 1
                base0 = j0 * VS + p * 130
                base1 = j1 * VS + p * 130
                last = (j1 == nk - 1)
                # avp is a single PSUM bank: exactly one start (first mm of
                # the bank) and one stop (last mm of the bank).
                nc.tensor.matmul(avp[:, 0:QC], v_sb[:, base0:base0 + 65],
                                 e_t[:, 0:QC],
                                 start=(j0 == 0), stop=False)
                nc.tensor.matmul(avp[:, 0:QC], v_sb[:, base1:base1 + 65],
                                 e_t[:, QC:2 * QC],
                                 start=False, stop=False)
                nc.tensor.matmul(avp[:, QC:2 * QC],
                                 v_sb[:, base0 + 65:base0 + 130],
                                 e_t[:, 2 * QC:3 * QC],
                                 start=False, stop=False)
                nc.tensor.matmul(avp[:, QC:2 * QC],
                                 v_sb[:, base1 + 65:base1 + 130],
                                 e_t[:, 3 * QC:4 * QC],
                                 start=False, stop=last)
                if last and cb is not None:
                    cb()

            def c_run(p, qc):
                nk = 2 * (qc + 1)
                avp = ps_av.tile([65, 2 * QC], F32, tag="av",
                                 name=f"av{p}_{qc}")
                qA = qT_sb[0:64, p * T + qc * QC:][:, :QC]
                qB = qT_sb[64:128, p * T + qc * QC:][:, :QC]

                def cb(pp=p, qq=qc, aa=avp):
                    norm(pp, qq, aa)
                    if pp == 3:
                        fill.append(proj_gen(qq))

                for u in range(qc + 1):
                    j0, j1 = 2 * u, 2 * u + 1
                    kt0 = kT_sb[:, p * T + j0 * P:][:, :P]
                    kt1 = kT_sb[:, p * T + j1 * P:][:, :P]
                    # sc spans 2 PSUM banks: head A cols [0:2QC] (bank 0),
                    # head B cols [2QC:4QC] (bank 1) -> concurrent row-group
                    # matmuls land in different banks; one start/stop per bank.
                    sc = ps_sc.tile([P, 4 * QC], F32, tag="sc")
                    nc.tensor.matmul(sc[:, 0:QC], kt0[0:64, :], qA,
                                     start=True, stop=False,
                                     tile_position=(0, 0))
                    nc.tensor.matmul(sc[:, 2 * QC:3 * QC], kt0[64:128, :],
                                     qB, start=True, stop=False,
                                     tile_position=(64, 0))
                    nc.tensor.matmul(sc[:, QC:2 * QC], kt1[0:64, :], qA,
                                     start=False, stop=True,
                                     tile_position=(0, 0))
                    nc.tensor.matmul(sc[:, 3 * QC:4 * QC], kt1[64:128, :],
                                     qB, start=False, stop=True,
                                     tile_position=(64, 0))
                    e_t = epool.tile([P, 4 * QC], BF, tag="exp")
                    nc.scalar.activation(e_t[:], sc[:], EXP, scale=SCALE)
                    if u == qc:
                        e_m = epool.tile([P, 4 * QC], BF, tag="expm", bufs=2)
                        nc.vector.tensor_mul(e_m[:], e_t[:], mask_sb[:])
                        e_t = e_m
                    # Fillers BEFORE the AV emission: if the pending unit's
                    # exp/mask isn't done yet, the filler matmuls keep the PE
                    # queue moving instead of stalling behind the AV wait.
                    pop_fill(2)
                    if len(pend) >= 2:
                        emit_av(pend.popleft())
                    pend.append((e_t, avp, p, j0, nk,
                                 cb if u == qc else None))

            fill.append(kv_gen())
            for qc in range(8):
                drain_until_tb(qc // 2)
                for p in range(NP):
                    c_run(p, qc)
            while pend:
                pop_fill(2)
                emit_av(pend.popleft())
            pop_fill(10 ** 6)
    return nc


def _make_mask():
    s = np.arange(P)[:, None]
    t = np.arange(QC)[None, :]
    m0 = (s <= t).astype(np.float32)
    m1 = (s <= t - P).astype(np.float32)
    return np.ascontiguousarray(
        np.concatenate([m0, m1, m0, m1], axis=1).astype(BF16NP)
    )


_CACHE = {}


def _get_nc():
    if "nc" not in _CACHE:
        nc = bacc.Bacc("TRN2", target_bir_lowering=False, debug=False)
        build_kernel(nc)
        nc.compile()
        _CACHE["nc"] = nc
    return _CACHE["nc"]


def make_in_maps(x, wq, wk, wv, w_proj, b_proj):
    x = np.asarray(x, np.float32)
    wq = np.asarray(wq, np.float32)
    wk = np.asarray(wk, np.float32)
    wv = np.asarray(wv, np.float32)
    w_proj = np.asarray(w_proj, np.float32)
    mask = _make_mask()

    halves = []
    for hh in range(2):
        hs = slice(hh * 8, hh * 8 + 8)
        halves.append({
            "wq": np.ascontiguousarray(
                np.transpose(wq[hs], (1, 0, 2)).reshape(C, HC).astype(BF16NP)),
            "wk": np.ascontiguousarray(
                np.transpose(wk[hs], (1, 0, 2)).reshape(C, HC).astype(BF16NP)),
            "wv": np.ascontiguousarray(
                np.transpose(wv[hs], (1, 0, 2)).reshape(C, HC).astype(BF16NP)),
            "wp": np.ascontiguousarray(
                w_proj[hh * HC:(hh + 1) * HC, :].astype(BF16NP)),
        })
    xTs = [np.ascontiguousarray(x[b].T.astype(BF16NP)) for b in range(B)]

    in_maps = []
    for core in range(8):
        b, hh = core // 2, core % 2
        w = halves[hh]
        in_maps.append({
            "xT": xTs[b],
            "wq": w["wq"], "wk": w["wk"], "wv": w["wv"], "wp": w["wp"],
            "maskd": mask,
        })
    return in_maps


def assemble(results, b_proj):
    bias = np.asarray(b_proj, np.float32)
    full = np.empty((B, T, C), np.float32)
    for b in range(B):
        p0 = np.asarray(results[2 * b]["out"]).astype(np.float32)
        p1 = np.asarray(results[2 * b + 1]["out"]).astype(np.float32)
        full[b] = p0 + p1 + bias[None, :]
    return full


def kernel(x, wq, wk, wv, w_proj, b_proj, _trace=False, _tmpdir=None):
    in_maps = make_in_maps(x, wq, wk, wv, w_proj, b_proj)
    nc = _get_nc()
    res = run_bass_kernel_spmd(
        nc, in_maps, core_ids=list(range(8)), trace=_trace, tmpdir=_tmpdir
    )
    if _trace:
        _CACHE["last_result"] = res
    return assemble(res.results, b_proj)
